# revision 61
# baseline (speedup 1.0000x reference)
"""Trainium2 Bass kernel for the GAT-style attention nn.Module.

Math: scores[b,i,j] = leaky_relu(sa_i + sb_j + bc) with sa = x@(Wa.T@wc_a)+ba.wc_a,
sb = x@(Wb.T@wc_b)+bb.wc_b.  Since exp(lrelu(t)) factorizes on each side of t=0
(exp(t)=E p_i q_j, exp(.01t)=E' p'_i q'_j) the softmax-weighted sum over keys
reduces to two masked sums over keys split at sb_j >= theta_i.  We bucketize sb
into K=128 quantized buckets, aggregate per-bucket sums of q*x (and q'*x) via a
one-hot matmul, project through Wv once per bucket, and resolve each query's
threshold with comparison-mask matmuls against the bucket tables.  Leaky-relu
continuity makes bucket-boundary misclassification error O(bucket width), so the
quantized split is numerically safe.  O(N*H + N*K*H/32) work instead of O(N^2*H).

Sharding: core c handles batch b=c//2, query half h=c%2.  Each core receives the
FULL batch's 4096 keys (host rolls x[b] so its 2048 queries are rows 0:2048) and
computes the bucket tables locally - no cross-core collective, so each core's
NEFF executes independently of the others' launch times.

x is shipped as bf16 (host-side dtype prep; halves the input DMA).  The score
dot products run on the DVE as x lands (every transpose route - XBAR DMA or PE
- costs ~11us/MB in descriptor or copy overhead, while the DVE starts at t~5us
with no staging).  Query-side sa runs first so the mask chain overlaps the
key-side sb dots; the one-hot -> bucket-sum matmuls interleave per 8-chunk
group.  Bucket counts ride a ones column in x.  All four strips' softmax
denominators come off the PE in one row; 1/den = exp(-ln(den)) on the ACT
tables, broadcast once, applied during the PSUM->SBUF copy of the numerators.
"""

import numpy as np

B, N, H = 4, 4096, 256
P = 128
NKCH = 32       # key chunks per core (full batch replicated)
QCH = 16        # query chunks
NQ = QCH * P    # 2048 queries per core
K = 128         # score buckets
NCORES = 8
NSTRIP = 4      # query strips of 512 for the lookup/mlp phase

_CACHE = {}


def _build(loop_n=None):
    import concourse.bacc as bacc
    import concourse.mybir as mybir
    from concourse.tile import TileContext
    from concourse.masks import make_identity

    F32 = mybir.dt.float32
    BF16 = mybir.dt.bfloat16
    I32 = mybir.dt.int32
    AF = mybir.ActivationFunctionType
    OP = mybir.AluOpType

    nc = bacc.Bacc("TRN2", target_bir_lowering=False, debug=False,
                   enable_asserts=False, num_devices=NCORES)

    xk_d = nc.dram_tensor("xk", [N, H], BF16, kind="ExternalInput")
    Wa_d = nc.dram_tensor("Wa", [H, H], F32, kind="ExternalInput")
    Wb_d = nc.dram_tensor("Wb", [H, H], F32, kind="ExternalInput")
    Wv_d = nc.dram_tensor("Wv", [H, H], F32, kind="ExternalInput")
    Wm_d = nc.dram_tensor("Wmlp", [H, H], F32, kind="ExternalInput")
    ba_d = nc.dram_tensor("ba", [H], F32, kind="ExternalInput")
    bb_d = nc.dram_tensor("bb", [H], F32, kind="ExternalInput")
    bv_d = nc.dram_tensor("bv", [H], F32, kind="ExternalInput")
    bm_d = nc.dram_tensor("bmlp", [H], F32, kind="ExternalInput")
    Wc_d = nc.dram_tensor("Wc", [1, 2 * H], F32, kind="ExternalInput")
    bc_d = nc.dram_tensor("bc", [1], F32, kind="ExternalInput")
    y_d = nc.dram_tensor("y", [NQ, H], F32, kind="ExternalOutput")

    xk_r = xk_d.ap().rearrange("(c p) f -> p c f", p=P)   # [128, 32, 256]
    y_r = y_d.ap().rearrange("(c p) f -> p c f", p=P)     # [128, 16, 256]

    with TileContext(nc) as tc:
        with tc.tile_pool(name="persist", bufs=1) as pp:

            import contextlib
            _loop = tc.For_i(0, loop_n, 1) if loop_n else contextlib.nullcontext()
            with _loop:
                # ---------- x row layout: query chunks on sync, keys after weights ----------
                xkb = pp.tile([P, NKCH, H + 1], BF16)
                nc.vector.memset(xkb[:, :, H:H + 1], 1.0)

                # wa/wca/wb/wcb gate the ua|ub row: they go FIRST, split over
                # both queues, then the x chunks, then the remaining weights
                wa_sb = pp.tile([P, 2, H], F32)
                wb_sb = pp.tile([P, 2, H], F32)
                wv_sb = pp.tile([P, 2, H], F32)
                wm_sb = pp.tile([P, 2, H], F32)
                wca = pp.tile([P, 2], F32)
                wcb = pp.tile([P, 2], F32)
                ba_c = pp.tile([P, 2], F32)
                bb_c = pp.tile([P, 2], F32)
                bm_c = pp.tile([P, 2], F32)
                bv_row = pp.tile([1, H], F32)
                bc_t = pp.tile([1, 1], F32)
                nc.sync.dma_start(out=wa_sb, in_=Wa_d.ap().rearrange("(c p) f -> p c f", p=P))
                nc.sync.dma_start(out=wca, in_=Wc_d.ap()[0:1, 0:H].rearrange("o (c p) -> p (o c)", p=P))
                nc.scalar.dma_start(out=wb_sb, in_=Wb_d.ap().rearrange("(c p) f -> p c f", p=P))
                nc.scalar.dma_start(out=wcb, in_=Wc_d.ap()[0:1, H:2 * H].rearrange("o (c p) -> p (o c)", p=P))
                for g in range(2):
                    nc.sync.dma_start(out=xkb[:, 8 * g:8 * g + 8, 0:H],
                                      in_=xk_r[:, 8 * g:8 * g + 8, :])
                    nc.scalar.dma_start(out=xkb[:, 16 + 8 * g:16 + 8 * g + 8, 0:H],
                                        in_=xk_r[:, 16 + 8 * g:16 + 8 * g + 8, :])
                nc.scalar.dma_start(out=ba_c, in_=ba_d.ap().rearrange("(c p) -> p c", p=P))
                nc.scalar.dma_start(out=bb_c, in_=bb_d.ap().rearrange("(c p) -> p c", p=P))
                nc.scalar.dma_start(out=bc_t, in_=bc_d.ap().rearrange("(o f) -> o f", o=1))
                nc.sync.dma_start(out=wv_sb, in_=Wv_d.ap().rearrange("(c p) f -> p c f", p=P))
                nc.sync.dma_start(out=wm_sb, in_=Wm_d.ap().rearrange("(c p) f -> p c f", p=P))
                nc.scalar.dma_start(out=bm_c, in_=bm_d.ap().rearrange("(c p) -> p c", p=P))
                nc.scalar.dma_start(out=bv_row, in_=bv_d.ap().rearrange("(o f) -> o f", o=1))

                # ---------- constants ----------
                iotac = pp.tile([P, 1], F32)            # value = partition idx
                nc.gpsimd.iota(iotac[:], pattern=[[0, 1]], base=0,
                               channel_multiplier=1,
                               allow_small_or_imprecise_dtypes=True)
                iota_b = pp.tile([P, K], BF16)          # value = bucket idx
                nc.gpsimd.iota(iota_b[:], pattern=[[1, K]], base=0,
                               channel_multiplier=0,
                               allow_small_or_imprecise_dtypes=True)
                identf = pp.tile([P, P], F32)
                identb = pp.tile([P, P], BF16)
                make_identity(nc, identf[:])
                make_identity(nc, identb[:])

                # ---------- init compute: ua/ub rows, scalars, wvT/wmT ----------
                uab_rowb = pp.tile([1, 2 * H], BF16)
                sc3_row = pp.tile([1, 3], F32)       # (ca, cb, bc)
                wvT = pp.tile([P, 2, H], BF16)   # Wv.T: [f_in, f_out]
                wmT = pp.tile([P, 2, H], BF16)   # Wmlp.T
                with tc.tile_pool(name="ps_u", bufs=1, space="PSUM") as ps_u, \
                     tc.tile_pool(name="ps_wt", bufs=1, space="PSUM") as ps_wt:
                    psu = ps_u.tile([1, 2 * H], F32, tag="psu")
                    for c in range(2):
                        nc.tensor.matmul(psu[0:1, 0:H], wca[:, c:c + 1], wa_sb[:, c, :],
                                         start=(c == 0), stop=(c == 1))
                    for c in range(2):
                        nc.tensor.matmul(psu[0:1, H:2 * H], wcb[:, c:c + 1], wb_sb[:, c, :],
                                         start=(c == 0), stop=(c == 1))
                    nc.vector.tensor_copy(out=uab_rowb, in_=psu)
                    psc = ps_u.tile([1, 2], F32, tag="psc")
                    for c in range(2):
                        nc.tensor.matmul(psc[0:1, 0:1], wca[:, c:c + 1], ba_c[:, c:c + 1],
                                         start=(c == 0), stop=(c == 1))
                    for c in range(2):
                        nc.tensor.matmul(psc[0:1, 1:2], wcb[:, c:c + 1], bb_c[:, c:c + 1],
                                         start=(c == 0), stop=(c == 1))
                    nc.vector.tensor_copy(out=sc3_row[0:1, 0:2], in_=psc)
                    nc.vector.tensor_copy(out=sc3_row[0:1, 2:3], in_=bc_t)

                    # weight transposes
                    for i in range(2):
                        for j in range(2):
                            pt = ps_wt.tile([P, P], F32, tag="wt")
                            nc.tensor.transpose(pt, wv_sb[:, i, j * P:(j + 1) * P], identf)
                            nc.scalar.copy(wvT[:, j, i * P:(i + 1) * P], pt)
                            pt2 = ps_wt.tile([P, P], F32, tag="wt2")
                            nc.tensor.transpose(pt2, wm_sb[:, i, j * P:(j + 1) * P], identf)
                            nc.vector.tensor_copy(out=wmT[:, j, i * P:(i + 1) * P], in_=pt2)

                # broadcasts (row already cast to bf16 by the psum copy)
                uab_b16 = pp.tile([P, 2 * H], BF16)
                nc.gpsimd.partition_broadcast(uab_b16[:], uab_rowb[:], channels=P)
                sc3 = pp.tile([P, 3], F32)           # cols: ca, cb, bc
                nc.gpsimd.partition_broadcast(sc3[:], sc3_row[:], channels=P)
                bv_bc = pp.tile([P, H], F32)
                nc.gpsimd.partition_broadcast(bv_bc[:], bv_row[:], channels=P)

                # ||ub||^2 as a per-partition self-dot of the broadcast row
                ubsq = pp.tile([P, 1], F32)
                with tc.tile_pool(name="scr0", bufs=2) as scr0:
                    d0 = scr0.tile([P, H], BF16, tag="d0")
                    nc.vector.scalar_tensor_tensor(
                        out=d0, in0=uab_b16[:, H:2 * H], scalar=0.0,
                        in1=uab_b16[:, H:2 * H], op0=OP.bypass, op1=OP.mult,
                        accum_out=ubsq)

                # ---------- query-side sa dots (DVE) ----------
                sah = pp.tile([P, QCH], F32)
                sbh = pp.tile([P, NKCH], F32)
                c_all = pp.tile([P, NKCH, K], BF16)
                with tc.tile_pool(name="scr", bufs=3) as scr:
                    for qc in range(QCH):
                        dsc = scr.tile([P, H], BF16, tag="dsc")
                        nc.vector.scalar_tensor_tensor(
                            out=dsc, in0=xkb[:, qc, 0:H], scalar=0.0,
                            in1=uab_b16[:, 0:H], op0=OP.bypass, op1=OP.mult,
                            accum_out=sah[:, qc:qc + 1])

                    # quantizer scalars (sig is already per-partition)
                    capbc = pp.tile([P, 1], F32)         # ca + bc
                    nc.vector.tensor_tensor(out=capbc, in0=sc3[:, 0:1],
                                            in1=sc3[:, 2:3], op=OP.add)
                    bias_pp = pp.tile([P, 1], F32)       # 0.01*(ca+bc)
                    nc.vector.tensor_scalar_mul(bias_pp, capbc, 0.01)
                    sig_col = pp.tile([P, 1], F32)
                    nc.scalar.activation(sig_col, ubsq, AF.Sqrt, bias=0.0, scale=1.0)
                    sig6 = pp.tile([P, 1], F32)          # 6.2 sigma
                    nc.vector.tensor_scalar_mul(sig6, sig_col, 6.2)
                    denom = pp.tile([P, 1], F32)         # full range = 12.4 sigma
                    nc.vector.tensor_scalar_mul(denom, sig_col, 12.4)
                    inv = pp.tile([P, 1], F32)
                    nc.vector.reciprocal(inv, denom)
                    scl = pp.tile([P, 1], F32)
                    nc.vector.tensor_scalar_mul(scl, inv, float(K))
                    nscl = pp.tile([P, 1], F32)
                    nc.vector.tensor_scalar_mul(nscl, scl, -1.0)
                    s1c = pp.tile([P, 1], F32)           # cb - lo_full = sig6
                    nc.vector.tensor_copy(out=s1c, in_=sig6)
                    lo_full = pp.tile([P, 1], F32)       # cb - sig6
                    nc.vector.tensor_tensor(out=lo_full, in0=sc3[:, 1:2], in1=sig6,
                                            op=OP.subtract)
                    s1d = pp.tile([P, 1], F32)           # ca + bc + lo_full
                    nc.vector.tensor_tensor(out=s1d, in0=capbc, in1=lo_full, op=OP.add)
                    w_col = pp.tile([P, 1], F32)
                    nc.vector.tensor_scalar_mul(w_col, denom, 1.0 / float(K))
                    ebias = pp.tile([P, 1], F32)     # lo_full + 0.5*w
                    nc.vector.tensor_scalar(out=ebias, in0=w_col, scalar1=0.5,
                                            scalar2=None, op0=OP.mult)
                    nc.vector.tensor_tensor(out=ebias, in0=ebias, in1=lo_full, op=OP.add)
                    e1_col = pp.tile([P, 1], F32)
                    e2_col = pp.tile([P, 1], F32)
                    ebias2 = pp.tile([P, 1], F32)
                    w2_col = pp.tile([P, 1], F32)
                    nc.vector.tensor_scalar_mul(ebias2, ebias, 0.01)
                    nc.vector.tensor_scalar_mul(w2_col, w_col, 0.01)
                    nc.scalar.activation(e1_col, iotac, AF.Exp, bias=ebias[:, 0:1],
                                         scale=w_col[:, 0:1])
                    nc.scalar.activation(e2_col, iotac, AF.Exp, bias=ebias2[:, 0:1],
                                         scale=w2_col[:, 0:1])
                    phat = pp.tile([P, QCH], F32)
                    phatp = pp.tile([P, QCH], F32)
                    nc.scalar.activation(phat, sah, AF.Exp, bias=capbc[:, 0:1], scale=1.0)
                    nc.scalar.activation(phatp, sah, AF.Exp, bias=bias_pp[:, 0:1],
                                         scale=0.01)

                    # query bucket index
                    d_f = pp.tile([P, QCH], F32)
                    d_i = pp.tile([P, QCH], I32)
                    nc.vector.tensor_scalar(out=d_f, in0=sah, scalar1=s1d[:, 0:1],
                                            scalar2=nscl[:, 0:1], op0=OP.add, op1=OP.mult)
                    nc.vector.tensor_scalar(out=d_f, in0=d_f, scalar1=-1.0,
                                            scalar2=float(K + 1), op0=OP.max, op1=OP.min)
                    nc.vector.tensor_copy(out=d_i, in_=d_f)
                    nc.vector.tensor_copy(out=d_f, in_=d_i)

                    # ---------- key-side sb dots + bucketize, per 8-chunk group ----------
                    c_f = pp.tile([P, NKCH], F32)
                    c_i = pp.tile([P, NKCH], I32)
                    for g in range(4):
                        for ci in range(8 * g, 8 * g + 8):
                            dsc = scr.tile([P, H], BF16, tag="dsc")
                            nc.vector.scalar_tensor_tensor(
                                out=dsc, in0=xkb[:, ci, 0:H], scalar=0.0,
                                in1=uab_b16[:, H:2 * H], op0=OP.bypass, op1=OP.mult,
                                accum_out=sbh[:, ci:ci + 1])
                        gs = slice(8 * g, 8 * g + 8)
                        nc.vector.tensor_scalar(out=c_f[:, gs], in0=sbh[:, gs],
                                                scalar1=s1c[:, 0:1], scalar2=scl[:, 0:1],
                                                op0=OP.add, op1=OP.mult)
                        nc.vector.tensor_scalar(out=c_f[:, gs], in0=c_f[:, gs],
                                                scalar1=0.0, scalar2=float(K - 1),
                                                op0=OP.max, op1=OP.min)
                        nc.vector.tensor_copy(out=c_i[:, gs], in_=c_f[:, gs])
                        nc.vector.tensor_copy(out=c_f[:, gs], in_=c_i[:, gs])
                        nc.vector.tensor_tensor(
                            out=c_all[:, gs, :],
                            in0=iota_b.unsqueeze(1).broadcast_to([P, 8, K]),
                            in1=c_f[:, gs].unsqueeze(2).broadcast_to([P, 8, K]),
                            op=OP.is_equal)

                # query masks fused with phat scaling (batched TTs)
                mge_p = pp.tile([P, QCH, K], BF16)
                mlt_p = pp.tile([P, QCH, K], BF16)
                iota_q = iota_b.unsqueeze(1).broadcast_to([P, QCH, K])
                nc.vector.tensor_tensor(
                    out=mge_p, in0=iota_q,
                    in1=d_f.unsqueeze(2).broadcast_to([P, QCH, K]), op=OP.is_ge)
                nc.vector.tensor_tensor(
                    out=mge_p, in0=mge_p,
                    in1=phat.unsqueeze(2).broadcast_to([P, QCH, K]), op=OP.mult)
                nc.vector.tensor_tensor(
                    out=mlt_p, in0=iota_q,
                    in1=d_f.unsqueeze(2).broadcast_to([P, QCH, K]), op=OP.is_lt)
                nc.vector.tensor_tensor(
                    out=mlt_p, in0=mlt_p,
                    in1=phatp.unsqueeze(2).broadcast_to([P, QCH, K]), op=OP.mult)
                fgeT = pp.tile([P, QCH, P], BF16)
                fltT = pp.tile([P, QCH, P], BF16)
                nc.sync.dma_start_transpose(out=fgeT[:], in_=mge_p[:])
                nc.scalar.dma_start_transpose(out=fltT[:], in_=mlt_p[:])

                # ---------- one-hot + bucket aggregation, interleaved per group ----------
                tabS = pp.tile([P, H], BF16)
                tabT = pp.tile([P, H], BF16)
                g1s = pp.tile([P, H + 1], BF16)    # bf16 so the count column
                g2s = pp.tile([P, H + 1], BF16)    # feeds pden's matmul directly
                rln = pp.tile([1, NSTRIP * 4 * P], F32)
                r_row = pp.tile([1, NSTRIP * 4 * P], F32)
                rbc = pp.tile([P, NSTRIP * 4 * P], F32)
                with tc.tile_pool(name="ps_g", bufs=1, space="PSUM") as ps_g, \
                     tc.tile_pool(name="ps_den", bufs=1, space="PSUM") as ps_den, \
                     tc.tile_pool(name="ps_t2", bufs=1, space="PSUM") as ps_t2, \
                     tc.tile_pool(name="ps_gv", bufs=1, space="PSUM") as ps_gv:
                    G1 = ps_g.tile([P, H + 1], F32, tag="G1")
                    for ci in range(NKCH):
                        nc.tensor.matmul(G1, c_all[:, ci, :], xkb[:, ci, :],
                                         start=(ci == 0), stop=(ci == NKCH - 1))
                    # q ~ const per bucket: row-scale raw sums by e1/e2
                    nc.vector.tensor_scalar(out=g1s, in0=G1, scalar1=e1_col[:, 0:1],
                                            scalar2=None, op0=OP.mult)
                    nc.vector.tensor_scalar(out=g2s, in0=G1, scalar1=e2_col[:, 0:1],
                                            scalar2=None, op0=OP.mult)

                    # all-strip denominators: one [1, 2048] row off the PE
                    pden = ps_den.tile([1, NSTRIP * 4 * P], F32, tag="pden")
                    for st in range(NSTRIP):
                        q0 = 4 * st
                        sl = slice(512 * st, 512 * (st + 1))
                        nc.tensor.matmul(pden[0:1, sl], g1s[:, H:H + 1],
                                         fgeT[:, q0:q0 + 4, :], start=True, stop=False)
                        nc.tensor.matmul(pden[0:1, sl], g2s[:, H:H + 1],
                                         fltT[:, q0:q0 + 4, :], start=False, stop=True)
                    # 1/den = exp(-ln(den)) on the ACT tables (the DVE
                    # reciprocal is ~6.5ns/elem and would serialize strips);
                    # broadcast per strip so attnT(0) starts sooner
                    nc.scalar.activation(rln, pden, AF.Ln, bias=0.0, scale=1.0)
                    nc.scalar.activation(r_row, rln, AF.Exp, bias=0.0, scale=-1.0)
                    for st in range(NSTRIP):
                        sl = slice(512 * st, 512 * (st + 1))
                        nc.gpsimd.partition_broadcast(rbc[:, sl], r_row[0:1, sl],
                                                      channels=P)

                    # transpose Gx and project through Wv.T
                    gxT1 = pp.tile([P, 2, K], BF16)
                    gxT2 = pp.tile([P, 2, K], BF16)
                    for j in range(2):
                        pt = ps_t2.tile([P, P], BF16, tag="tp")
                        nc.tensor.transpose(pt, g1s[:, j * P:(j + 1) * P], identb)
                        nc.scalar.copy(gxT1[:, j, :], pt)
                        pt2 = ps_t2.tile([P, P], BF16, tag="tp")
                        nc.tensor.transpose(pt2, g2s[:, j * P:(j + 1) * P], identb)
                        nc.scalar.copy(gxT2[:, j, :], pt2)
                    Gv1 = ps_gv.tile([P, H], F32, tag="Gv1")
                    Gv2 = ps_gv.tile([P, H], F32, tag="Gv2")
                    for j in range(2):
                        nc.tensor.matmul(Gv1, gxT1[:, j, :], wvT[:, j, :],
                                         start=(j == 0), stop=(j == 1))
                    for j in range(2):
                        nc.tensor.matmul(Gv2, gxT2[:, j, :], wvT[:, j, :],
                                         start=(j == 0), stop=(j == 1))
                    # tab = Gv + gq * bv   (outer product via per-partition scalar)
                    nc.vector.scalar_tensor_tensor(out=tabS, in0=bv_bc,
                                                   scalar=g1s[:, H:H + 1], in1=Gv1,
                                                   op0=OP.mult, op1=OP.add)
                    nc.vector.scalar_tensor_tensor(out=tabT, in0=bv_bc,
                                                   scalar=g2s[:, H:H + 1], in1=Gv2,
                                                   op0=OP.mult, op1=OP.add)

                # ---------- query tail, software-pipelined strips of 512 ----------
                def _pnum(ps_num, st):
                    q0 = 4 * st
                    pn = ps_num.tile([P, 2, 512], F32, tag="pnum")
                    for m in range(2):
                        nc.tensor.matmul(pn[:, m, :], tabS[:, m * P:(m + 1) * P],
                                         fgeT[:, q0:q0 + 4, :], start=True, stop=False)
                        nc.tensor.matmul(pn[:, m, :], tabT[:, m * P:(m + 1) * P],
                                         fltT[:, q0:q0 + 4, :], start=False, stop=True)
                    return pn

                with tc.tile_pool(name="ps_num", bufs=3, space="PSUM") as ps_num, \
                     tc.tile_pool(name="ps_y", bufs=2, space="PSUM") as ps_y, \
                     tc.tile_pool(name="strip", bufs=3) as sp:
                    pnum = _pnum(ps_num, 0)
                    for st in range(NSTRIP):
                        q0 = 4 * st
                        # attn = num / den, fused into the PSUM->SBUF copy
                        attnT = sp.tile([P, 2, 512], BF16, tag="attnT")
                        for m in range(2):
                            nc.vector.scalar_tensor_tensor(
                                out=attnT[:, m, :], in0=pnum[:, m, :], scalar=0.0,
                                in1=rbc[:, 512 * st:512 * (st + 1)],
                                op0=OP.bypass, op1=OP.mult)
                        # keep the PE streaming: next strip's pnum before pz
                        if st + 1 < NSTRIP:
                            pnum = _pnum(ps_num, st + 1)

                        pz = ps_num.tile([P, 2, 512], F32, tag="pnum")
                        for mo in range(2):
                            nc.tensor.matmul(pz[:, mo, :],
                                             wmT[:, 0, mo * P:(mo + 1) * P],
                                             attnT[:, 0, :], start=True, stop=False)
                            nc.tensor.matmul(pz[:, mo, :],
                                             wmT[:, 1, mo * P:(mo + 1) * P],
                                             attnT[:, 1, :], start=False, stop=True)
                        yt = sp.tile([P, 2, 512], BF16, tag="yt")
                        for mo in range(2):
                            nc.scalar.activation(yt[:, mo, :], pz[:, mo, :], AF.Tanh,
                                                 bias=bm_c[:, mo:mo + 1], scale=1.0)

                        # transpose y back to query-partition layout on the PE
                        py = ps_y.tile([P, 4, H], BF16, tag="py")
                        for qq in range(4):
                            for fc in range(2):
                                nc.tensor.transpose(py[:, qq, fc * P:(fc + 1) * P],
                                                    yt[:, fc, qq * P:(qq + 1) * P],
                                                    identb)
                        yout = sp.tile([P, 4, H], F32, tag="yout")
                        nc.vector.tensor_tensor(out=yout, in0=py,
                                                in1=xkb[:, q0:q0 + 4, 0:H], op=OP.add)
                        eng = nc.sync if st % 2 == 0 else nc.scalar
                        eng.dma_start(out=y_r[:, q0:q0 + 4, :], in_=yout)

    nc.compile()
    return nc


def _get_nc():
    if "nc" not in _CACHE:
        _CACHE["nc"] = _build()
    return _CACHE["nc"]


def _in_maps(np_inputs):
    import ml_dtypes
    x = np.asarray(np_inputs["x"], dtype=np.float32)
    w = {}
    for k in ("Wa", "Wb", "Wv", "Wmlp", "ba", "bb", "bv", "bmlp", "Wc", "bc"):
        w[k] = np.ascontiguousarray(np.asarray(np_inputs[k], np.float32))
    in_maps = []
    for c in range(NCORES):
        b, h = divmod(c, 2)
        m = dict(w)
        # full batch of keys, rolled so this core's queries are rows 0:NQ
        m["xk"] = np.ascontiguousarray(
            np.concatenate([x[b, h * NQ:], x[b, :h * NQ]],
                           axis=0).astype(ml_dtypes.bfloat16))
        in_maps.append(m)
    return in_maps


def kernel(x, Wa, ba, Wb, bb, Wv, bv, Wc, bc, Wmlp, bmlp):
    from concourse.bass_utils import run_bass_kernel_spmd

    nc = _get_nc()
    in_maps = _in_maps(dict(x=x, Wa=Wa, ba=ba, Wb=Wb, bb=bb, Wv=Wv, bv=bv,
                            Wc=Wc, bc=bc, Wmlp=Wmlp, bmlp=bmlp))
    res = run_bass_kernel_spmd(nc, in_maps, core_ids=list(range(NCORES)))
    out = np.empty((B, N, H), np.float32)
    for c in range(NCORES):
        b, h = divmod(c, 2)
        out[b, h * NQ:(h + 1) * NQ] = res.results[c]["y"]
    return out


# revision 66
# speedup vs baseline: 1.0154x; 1.0154x over previous
"""Trainium2 Bass kernel for the GAT-style attention nn.Module.

Math: scores[b,i,j] = leaky_relu(sa_i + sb_j + bc) with sa = x@(Wa.T@wc_a)+ba.wc_a,
sb = x@(Wb.T@wc_b)+bb.wc_b.  Since exp(lrelu(t)) factorizes on each side of t=0
(exp(t)=E p_i q_j, exp(.01t)=E' p'_i q'_j) the softmax-weighted sum over keys
reduces to two masked sums over keys split at sb_j >= theta_i.  We bucketize sb
into K=128 quantized buckets, aggregate per-bucket sums of q*x (and q'*x) via a
one-hot matmul, project through Wv once per bucket, and resolve each query's
threshold with comparison-mask matmuls against the bucket tables.  Leaky-relu
continuity makes bucket-boundary misclassification error O(bucket width), so the
quantized split is numerically safe.  O(N*H + N*K*H/32) work instead of O(N^2*H).

Sharding: core c handles batch b=c//2, query half h=c%2.  Each core receives the
FULL batch's 4096 keys (host rolls x[b] so its 2048 queries are rows 0:2048) and
computes the bucket tables locally - no cross-core collective, so each core's
NEFF executes independently of the others' launch times.

x is shipped as bf16 (host-side dtype prep; halves the input DMA).  The score
dot products run on the DVE as x lands (every transpose route - XBAR DMA or PE
- costs ~11us/MB in descriptor or copy overhead, while the DVE starts at t~5us
with no staging).  Query-side sa runs first so the mask chain overlaps the
key-side sb dots; the one-hot -> bucket-sum matmuls interleave per 8-chunk
group.  Bucket counts ride a ones column in x.  All four strips' softmax
denominators come off the PE in one row; 1/den = exp(-ln(den)) on the ACT
tables, broadcast once, applied during the PSUM->SBUF copy of the numerators.
"""

import numpy as np

B, N, H = 4, 4096, 256
P = 128
NKCH = 32       # key chunks per core (full batch replicated)
QCH = 16        # query chunks
NQ = QCH * P    # 2048 queries per core
K = 128         # score buckets
NCORES = 8
NSTRIP = 4      # query strips of 512 for the lookup/mlp phase

_CACHE = {}


def _build(loop_n=None):
    import concourse.bacc as bacc
    import concourse.mybir as mybir
    from concourse.tile import TileContext
    from concourse.masks import make_identity

    F32 = mybir.dt.float32
    BF16 = mybir.dt.bfloat16
    I32 = mybir.dt.int32
    AF = mybir.ActivationFunctionType
    OP = mybir.AluOpType

    nc = bacc.Bacc("TRN2", target_bir_lowering=False, debug=False,
                   enable_asserts=False, num_devices=NCORES)

    xk_d = nc.dram_tensor("xk", [N, H], BF16, kind="ExternalInput")
    Wa_d = nc.dram_tensor("Wa", [H, H], F32, kind="ExternalInput")
    Wb_d = nc.dram_tensor("Wb", [H, H], F32, kind="ExternalInput")
    Wv_d = nc.dram_tensor("Wv", [H, H], F32, kind="ExternalInput")
    Wm_d = nc.dram_tensor("Wmlp", [H, H], F32, kind="ExternalInput")
    ba_d = nc.dram_tensor("ba", [H], F32, kind="ExternalInput")
    bb_d = nc.dram_tensor("bb", [H], F32, kind="ExternalInput")
    bv_d = nc.dram_tensor("bv", [H], F32, kind="ExternalInput")
    bm_d = nc.dram_tensor("bmlp", [H], F32, kind="ExternalInput")
    Wc_d = nc.dram_tensor("Wc", [1, 2 * H], F32, kind="ExternalInput")
    bc_d = nc.dram_tensor("bc", [1], F32, kind="ExternalInput")
    y_d = nc.dram_tensor("y", [NQ, H], F32, kind="ExternalOutput")

    xk_r = xk_d.ap().rearrange("(c p) f -> p c f", p=P)   # [128, 32, 256]
    y_r = y_d.ap().rearrange("(c p) f -> p c f", p=P)     # [128, 16, 256]

    with TileContext(nc) as tc:
        with tc.tile_pool(name="persist", bufs=1) as pp:

            import contextlib
            _loop = tc.For_i(0, loop_n, 1) if loop_n else contextlib.nullcontext()
            with _loop:
                # ---------- x row layout: query chunks on sync, keys after weights ----------
                xkb = pp.tile([P, NKCH, H + 1], BF16)
                nc.vector.memset(xkb[:, :, H:H + 1], 1.0)

                # wa/wca/wb/wcb gate the ua|ub row: they go FIRST, split over
                # both queues, then the x chunks, then the remaining weights
                wa_sb = pp.tile([P, 2, H], F32)
                wb_sb = pp.tile([P, 2, H], F32)
                wv_sb = pp.tile([P, 2, H], F32)
                wm_sb = pp.tile([P, 2, H], F32)
                wca = pp.tile([P, 2], F32)
                wcb = pp.tile([P, 2], F32)
                ba_c = pp.tile([P, 2], F32)
                bb_c = pp.tile([P, 2], F32)
                bm_c = pp.tile([P, 2], F32)
                bv_row = pp.tile([1, H], F32)
                bc_t = pp.tile([1, 1], F32)
                nc.sync.dma_start(out=wa_sb, in_=Wa_d.ap().rearrange("(c p) f -> p c f", p=P))
                nc.sync.dma_start(out=wca, in_=Wc_d.ap()[0:1, 0:H].rearrange("o (c p) -> p (o c)", p=P))
                nc.scalar.dma_start(out=wb_sb, in_=Wb_d.ap().rearrange("(c p) f -> p c f", p=P))
                nc.scalar.dma_start(out=wcb, in_=Wc_d.ap()[0:1, H:2 * H].rearrange("o (c p) -> p (o c)", p=P))
                for g in range(2):
                    nc.sync.dma_start(out=xkb[:, 8 * g:8 * g + 8, 0:H],
                                      in_=xk_r[:, 8 * g:8 * g + 8, :])
                    nc.scalar.dma_start(out=xkb[:, 16 + 8 * g:16 + 8 * g + 8, 0:H],
                                        in_=xk_r[:, 16 + 8 * g:16 + 8 * g + 8, :])
                nc.scalar.dma_start(out=ba_c, in_=ba_d.ap().rearrange("(c p) -> p c", p=P))
                nc.scalar.dma_start(out=bb_c, in_=bb_d.ap().rearrange("(c p) -> p c", p=P))
                nc.scalar.dma_start(out=bc_t, in_=bc_d.ap().rearrange("(o f) -> o f", o=1))
                nc.sync.dma_start(out=wv_sb, in_=Wv_d.ap().rearrange("(c p) f -> p c f", p=P))
                nc.sync.dma_start(out=wm_sb, in_=Wm_d.ap().rearrange("(c p) f -> p c f", p=P))
                nc.scalar.dma_start(out=bm_c, in_=bm_d.ap().rearrange("(c p) -> p c", p=P))
                nc.scalar.dma_start(out=bv_row, in_=bv_d.ap().rearrange("(o f) -> o f", o=1))

                # ---------- constants ----------
                iotac = pp.tile([P, 1], F32)            # value = partition idx
                nc.gpsimd.iota(iotac[:], pattern=[[0, 1]], base=0,
                               channel_multiplier=1,
                               allow_small_or_imprecise_dtypes=True)
                iota_b = pp.tile([P, K], BF16)          # value = bucket idx
                nc.gpsimd.iota(iota_b[:], pattern=[[1, K]], base=0,
                               channel_multiplier=0,
                               allow_small_or_imprecise_dtypes=True)
                identf = pp.tile([P, P], F32)
                identb = pp.tile([P, P], BF16)
                make_identity(nc, identf[:])

                # ---------- init compute: ua/ub rows, scalars, wvT/wmT ----------
                uab_rowb = pp.tile([1, 2 * H], BF16)
                sc3_row = pp.tile([1, 3], F32)       # (ca, cb, bc)
                wvT = pp.tile([P, 2, H], F32)    # Wv.T: [f_in, f_out]
                wmT = pp.tile([P, 2, H], BF16)   # Wmlp.T
                with tc.tile_pool(name="ps_u", bufs=1, space="PSUM") as ps_u, \
                     tc.tile_pool(name="ps_wt", bufs=1, space="PSUM") as ps_wt:
                    psu = ps_u.tile([1, 2 * H], F32, tag="psu")
                    for c in range(2):
                        nc.tensor.matmul(psu[0:1, 0:H], wca[:, c:c + 1], wa_sb[:, c, :],
                                         start=(c == 0), stop=(c == 1))
                    for c in range(2):
                        nc.tensor.matmul(psu[0:1, H:2 * H], wcb[:, c:c + 1], wb_sb[:, c, :],
                                         start=(c == 0), stop=(c == 1))
                    nc.vector.tensor_copy(out=uab_rowb, in_=psu)
                    psc = ps_u.tile([1, 2], F32, tag="psc")
                    for c in range(2):
                        nc.tensor.matmul(psc[0:1, 0:1], wca[:, c:c + 1], ba_c[:, c:c + 1],
                                         start=(c == 0), stop=(c == 1))
                    for c in range(2):
                        nc.tensor.matmul(psc[0:1, 1:2], wcb[:, c:c + 1], bb_c[:, c:c + 1],
                                         start=(c == 0), stop=(c == 1))
                    nc.vector.tensor_copy(out=sc3_row[0:1, 0:2], in_=psc)
                    nc.vector.tensor_copy(out=sc3_row[0:1, 2:3], in_=bc_t)

                    # weight transposes
                    for i in range(2):
                        for j in range(2):
                            pt = ps_wt.tile([P, P], F32, tag="wt")
                            nc.tensor.transpose(pt, wv_sb[:, i, j * P:(j + 1) * P], identf)
                            nc.scalar.copy(wvT[:, j, i * P:(i + 1) * P], pt)
                            pt2 = ps_wt.tile([P, P], F32, tag="wt2")
                            nc.tensor.transpose(pt2, wm_sb[:, i, j * P:(j + 1) * P], identf)
                            nc.vector.tensor_copy(out=wmT[:, j, i * P:(i + 1) * P], in_=pt2)

                # broadcasts (row already cast to bf16 by the psum copy)
                uab_b16 = pp.tile([P, 2 * H], BF16)
                nc.gpsimd.partition_broadcast(uab_b16[:], uab_rowb[:], channels=P)
                sc3 = pp.tile([P, 3], F32)           # cols: ca, cb, bc
                nc.gpsimd.partition_broadcast(sc3[:], sc3_row[:], channels=P)
                bv_bc = pp.tile([P, H], F32)
                nc.gpsimd.partition_broadcast(bv_bc[:], bv_row[:], channels=P)
                # identb built AFTER the broadcasts (gpsimd work, first used
                # by the strip-phase transposes emitted much later)
                make_identity(nc, identb[:])

                # ||ub||^2 as a per-partition self-dot of the broadcast row
                ubsq = pp.tile([P, 1], F32)
                with tc.tile_pool(name="scr0", bufs=2) as scr0:
                    d0 = scr0.tile([P, H], BF16, tag="d0")
                    nc.vector.scalar_tensor_tensor(
                        out=d0, in0=uab_b16[:, H:2 * H], scalar=0.0,
                        in1=uab_b16[:, H:2 * H], op0=OP.bypass, op1=OP.mult,
                        accum_out=ubsq)

                # ---------- query-side sa dots (DVE) ----------
                sah = pp.tile([P, QCH], F32)
                sbh = pp.tile([P, NKCH], F32)
                c_all = pp.tile([P, NKCH, K], BF16)
                with tc.tile_pool(name="scr", bufs=3) as scr:
                    for qc in range(QCH):
                        dsc = scr.tile([P, H], BF16, tag="dsc")
                        nc.vector.scalar_tensor_tensor(
                            out=dsc, in0=xkb[:, qc, 0:H], scalar=0.0,
                            in1=uab_b16[:, 0:H], op0=OP.bypass, op1=OP.mult,
                            accum_out=sah[:, qc:qc + 1])

                    # quantizer scalars (sig is already per-partition)
                    capbc = pp.tile([P, 1], F32)         # ca + bc
                    nc.vector.tensor_tensor(out=capbc, in0=sc3[:, 0:1],
                                            in1=sc3[:, 2:3], op=OP.add)
                    bias_pp = pp.tile([P, 1], F32)       # 0.01*(ca+bc)
                    nc.vector.tensor_scalar_mul(bias_pp, capbc, 0.01)
                    sig_col = pp.tile([P, 1], F32)
                    nc.scalar.activation(sig_col, ubsq, AF.Sqrt, bias=0.0, scale=1.0)
                    sig6 = pp.tile([P, 1], F32)          # 6.2 sigma
                    nc.vector.tensor_scalar_mul(sig6, sig_col, 6.2)
                    denom = pp.tile([P, 1], F32)         # full range = 12.4 sigma
                    nc.vector.tensor_scalar_mul(denom, sig_col, 12.4)
                    inv = pp.tile([P, 1], F32)
                    nc.vector.reciprocal(inv, denom)
                    scl = pp.tile([P, 1], F32)
                    nc.vector.tensor_scalar_mul(scl, inv, float(K))
                    nscl = pp.tile([P, 1], F32)
                    nc.vector.tensor_scalar_mul(nscl, scl, -1.0)
                    s1c = pp.tile([P, 1], F32)           # cb - lo_full = sig6
                    nc.vector.tensor_copy(out=s1c, in_=sig6)
                    lo_full = pp.tile([P, 1], F32)       # cb - sig6
                    nc.vector.tensor_tensor(out=lo_full, in0=sc3[:, 1:2], in1=sig6,
                                            op=OP.subtract)
                    s1d = pp.tile([P, 1], F32)           # ca + bc + lo_full
                    nc.vector.tensor_tensor(out=s1d, in0=capbc, in1=lo_full, op=OP.add)
                    w_col = pp.tile([P, 1], F32)
                    nc.vector.tensor_scalar_mul(w_col, denom, 1.0 / float(K))
                    ebias = pp.tile([P, 1], F32)     # lo_full + 0.5*w
                    nc.vector.tensor_scalar(out=ebias, in0=w_col, scalar1=0.5,
                                            scalar2=None, op0=OP.mult)
                    nc.vector.tensor_tensor(out=ebias, in0=ebias, in1=lo_full, op=OP.add)
                    e1_col = pp.tile([P, 1], F32)
                    e2_col = pp.tile([P, 1], F32)
                    ebias2 = pp.tile([P, 1], F32)
                    w2_col = pp.tile([P, 1], F32)
                    nc.vector.tensor_scalar_mul(ebias2, ebias, 0.01)
                    nc.vector.tensor_scalar_mul(w2_col, w_col, 0.01)
                    nc.scalar.activation(e1_col, iotac, AF.Exp, bias=ebias[:, 0:1],
                                         scale=w_col[:, 0:1])
                    nc.scalar.activation(e2_col, iotac, AF.Exp, bias=ebias2[:, 0:1],
                                         scale=w2_col[:, 0:1])
                    phat = pp.tile([P, QCH], F32)
                    phatp = pp.tile([P, QCH], F32)
                    nc.scalar.activation(phat, sah, AF.Exp, bias=capbc[:, 0:1], scale=1.0)
                    nc.scalar.activation(phatp, sah, AF.Exp, bias=bias_pp[:, 0:1],
                                         scale=0.01)

                    # query bucket index
                    d_f = pp.tile([P, QCH], F32)
                    d_i = pp.tile([P, QCH], I32)
                    nc.vector.tensor_scalar(out=d_f, in0=sah, scalar1=s1d[:, 0:1],
                                            scalar2=nscl[:, 0:1], op0=OP.add, op1=OP.mult)
                    nc.vector.tensor_scalar(out=d_f, in0=d_f, scalar1=-1.0,
                                            scalar2=float(K + 1), op0=OP.max, op1=OP.min)
                    nc.vector.tensor_copy(out=d_i, in_=d_f)
                    nc.vector.tensor_copy(out=d_f, in_=d_i)

                    # ---------- key-side sb dots + bucketize, per 8-chunk group ----------
                    c_f = pp.tile([P, NKCH], F32)
                    c_i = pp.tile([P, NKCH], I32)
                    for g in range(4):
                        for ci in range(8 * g, 8 * g + 8):
                            dsc = scr.tile([P, H], BF16, tag="dsc")
                            nc.vector.scalar_tensor_tensor(
                                out=dsc, in0=xkb[:, ci, 0:H], scalar=0.0,
                                in1=uab_b16[:, H:2 * H], op0=OP.bypass, op1=OP.mult,
                                accum_out=sbh[:, ci:ci + 1])
                        gs = slice(8 * g, 8 * g + 8)
                        nc.vector.tensor_scalar(out=c_f[:, gs], in0=sbh[:, gs],
                                                scalar1=s1c[:, 0:1], scalar2=scl[:, 0:1],
                                                op0=OP.add, op1=OP.mult)
                        nc.vector.tensor_scalar(out=c_f[:, gs], in0=c_f[:, gs],
                                                scalar1=0.0, scalar2=float(K - 1),
                                                op0=OP.max, op1=OP.min)
                        nc.vector.tensor_copy(out=c_i[:, gs], in_=c_f[:, gs])
                        nc.vector.tensor_copy(out=c_f[:, gs], in_=c_i[:, gs])
                        nc.vector.tensor_tensor(
                            out=c_all[:, gs, :],
                            in0=iota_b.unsqueeze(1).broadcast_to([P, 8, K]),
                            in1=c_f[:, gs].unsqueeze(2).broadcast_to([P, 8, K]),
                            op=OP.is_equal)

                # query masks fused with phat scaling (batched TTs)
                mge_p = pp.tile([P, QCH, K], BF16)
                mlt_p = pp.tile([P, QCH, K], BF16)
                iota_q = iota_b.unsqueeze(1).broadcast_to([P, QCH, K])
                nc.vector.tensor_tensor(
                    out=mge_p, in0=iota_q,
                    in1=d_f.unsqueeze(2).broadcast_to([P, QCH, K]), op=OP.is_ge)
                nc.vector.tensor_tensor(
                    out=mge_p, in0=mge_p,
                    in1=phat.unsqueeze(2).broadcast_to([P, QCH, K]), op=OP.mult)
                nc.vector.tensor_tensor(
                    out=mlt_p, in0=iota_q,
                    in1=d_f.unsqueeze(2).broadcast_to([P, QCH, K]), op=OP.is_lt)
                nc.vector.tensor_tensor(
                    out=mlt_p, in0=mlt_p,
                    in1=phatp.unsqueeze(2).broadcast_to([P, QCH, K]), op=OP.mult)
                fgeT = pp.tile([P, QCH, P], BF16)
                fltT = pp.tile([P, QCH, P], BF16)
                nc.sync.dma_start_transpose(out=fgeT[:], in_=mge_p[:])
                nc.scalar.dma_start_transpose(out=fltT[:], in_=mlt_p[:])

                # ---------- one-hot + bucket aggregation, interleaved per group ----------
                tabS = pp.tile([P, H], BF16)
                tabT = pp.tile([P, H], BF16)
                g1s = pp.tile([P, H + 1], F32)
                g2s = pp.tile([P, H + 1], F32)
                gq_colb = pp.tile([P, 2], BF16)        # e-scaled bucket counts
                rln = pp.tile([1, NSTRIP * 4 * P], F32)
                r_row = pp.tile([1, NSTRIP * 4 * P], F32)
                rbc = pp.tile([P, NSTRIP * 4 * P], F32)
                with tc.tile_pool(name="ps_g", bufs=1, space="PSUM") as ps_g, \
                     tc.tile_pool(name="ps_den", bufs=1, space="PSUM") as ps_den, \
                     tc.tile_pool(name="ps_t2", bufs=1, space="PSUM") as ps_t2, \
                     tc.tile_pool(name="ps_gv", bufs=1, space="PSUM") as ps_gv:
                    G1 = ps_g.tile([P, H + 1], F32, tag="G1")
                    for ci in range(NKCH):
                        nc.tensor.matmul(G1, c_all[:, ci, :], xkb[:, ci, :],
                                         start=(ci == 0), stop=(ci == NKCH - 1))
                    # q ~ const per bucket: row-scale raw sums by e1/e2
                    nc.vector.tensor_scalar(out=g1s, in0=G1, scalar1=e1_col[:, 0:1],
                                            scalar2=None, op0=OP.mult)
                    nc.vector.tensor_scalar(out=g2s, in0=G1, scalar1=e2_col[:, 0:1],
                                            scalar2=None, op0=OP.mult)
                    nc.vector.tensor_copy(out=gq_colb[:, 0:1], in_=g1s[:, H:H + 1])
                    nc.vector.tensor_copy(out=gq_colb[:, 1:2], in_=g2s[:, H:H + 1])

                    # all-strip denominators: one [1, 2048] row off the PE
                    pden = ps_den.tile([1, NSTRIP * 4 * P], F32, tag="pden")
                    for st in range(NSTRIP):
                        q0 = 4 * st
                        sl = slice(512 * st, 512 * (st + 1))
                        nc.tensor.matmul(pden[0:1, sl], gq_colb[:, 0:1],
                                         fgeT[:, q0:q0 + 4, :], start=True, stop=False)
                        nc.tensor.matmul(pden[0:1, sl], gq_colb[:, 1:2],
                                         fltT[:, q0:q0 + 4, :], start=False, stop=True)
                    # 1/den = exp(-ln(den)) on the ACT tables (the DVE
                    # reciprocal is ~6.5ns/elem and would serialize strips);
                    # broadcast per strip so attnT(0) starts sooner
                    nc.scalar.activation(rln, pden, AF.Ln, bias=0.0, scale=1.0)
                    nc.scalar.activation(r_row, rln, AF.Exp, bias=0.0, scale=-1.0)
                    for st in range(NSTRIP):
                        sl = slice(512 * st, 512 * (st + 1))
                        nc.gpsimd.partition_broadcast(rbc[:, sl], r_row[0:1, sl],
                                                      channels=P)

                    # transpose Gx and project through Wv.T
                    gxT1 = pp.tile([P, 2, K], F32)
                    gxT2 = pp.tile([P, 2, K], F32)
                    for j in range(2):
                        pt = ps_t2.tile([P, P], F32, tag="tp")
                        nc.tensor.transpose(pt, g1s[:, j * P:(j + 1) * P], identf)
                        nc.scalar.copy(gxT1[:, j, :], pt)
                        pt2 = ps_t2.tile([P, P], F32, tag="tp")
                        nc.tensor.transpose(pt2, g2s[:, j * P:(j + 1) * P], identf)
                        nc.scalar.copy(gxT2[:, j, :], pt2)
                    Gv1 = ps_gv.tile([P, H], F32, tag="Gv1")
                    Gv2 = ps_gv.tile([P, H], F32, tag="Gv2")
                    for j in range(2):
                        nc.tensor.matmul(Gv1, gxT1[:, j, :], wvT[:, j, :],
                                         start=(j == 0), stop=(j == 1))
                    for j in range(2):
                        nc.tensor.matmul(Gv2, gxT2[:, j, :], wvT[:, j, :],
                                         start=(j == 0), stop=(j == 1))
                    # tab = Gv + gq * bv   (outer product via per-partition scalar)
                    nc.vector.scalar_tensor_tensor(out=tabS, in0=bv_bc,
                                                   scalar=g1s[:, H:H + 1], in1=Gv1,
                                                   op0=OP.mult, op1=OP.add)
                    nc.vector.scalar_tensor_tensor(out=tabT, in0=bv_bc,
                                                   scalar=g2s[:, H:H + 1], in1=Gv2,
                                                   op0=OP.mult, op1=OP.add)

                # ---------- query tail, software-pipelined strips of 512 ----------
                def _pnum(ps_num, st):
                    q0 = 4 * st
                    pn = ps_num.tile([P, 2, 512], F32, tag="pnum")
                    for m in range(2):
                        nc.tensor.matmul(pn[:, m, :], tabS[:, m * P:(m + 1) * P],
                                         fgeT[:, q0:q0 + 4, :], start=True, stop=False)
                        nc.tensor.matmul(pn[:, m, :], tabT[:, m * P:(m + 1) * P],
                                         fltT[:, q0:q0 + 4, :], start=False, stop=True)
                    return pn

                with tc.tile_pool(name="ps_num", bufs=3, space="PSUM") as ps_num, \
                     tc.tile_pool(name="ps_y", bufs=2, space="PSUM") as ps_y, \
                     tc.tile_pool(name="strip", bufs=3) as sp:
                    pnum = _pnum(ps_num, 0)
                    for st in range(NSTRIP):
                        q0 = 4 * st
                        # attn = num / den, fused into the PSUM->SBUF copy
                        attnT = sp.tile([P, 2, 512], BF16, tag="attnT")
                        for m in range(2):
                            nc.vector.scalar_tensor_tensor(
                                out=attnT[:, m, :], in0=pnum[:, m, :], scalar=0.0,
                                in1=rbc[:, 512 * st:512 * (st + 1)],
                                op0=OP.bypass, op1=OP.mult)
                        # keep the PE streaming: next strip's pnum before pz
                        if st + 1 < NSTRIP:
                            pnum = _pnum(ps_num, st + 1)

                        pz = ps_num.tile([P, 2, 512], F32, tag="pnum")
                        for mo in range(2):
                            nc.tensor.matmul(pz[:, mo, :],
                                             wmT[:, 0, mo * P:(mo + 1) * P],
                                             attnT[:, 0, :], start=True, stop=False)
                            nc.tensor.matmul(pz[:, mo, :],
                                             wmT[:, 1, mo * P:(mo + 1) * P],
                                             attnT[:, 1, :], start=False, stop=True)
                        yt = sp.tile([P, 2, 512], BF16, tag="yt")
                        for mo in range(2):
                            nc.scalar.activation(yt[:, mo, :], pz[:, mo, :], AF.Tanh,
                                                 bias=bm_c[:, mo:mo + 1], scale=1.0)

                        # transpose y back to query-partition layout on the PE
                        py = ps_y.tile([P, 4, H], BF16, tag="py")
                        for qq in range(4):
                            for fc in range(2):
                                nc.tensor.transpose(py[:, qq, fc * P:(fc + 1) * P],
                                                    yt[:, fc, qq * P:(qq + 1) * P],
                                                    identb)
                        yout = sp.tile([P, 4, H], F32, tag="yout")
                        nc.vector.tensor_tensor(out=yout, in0=py,
                                                in1=xkb[:, q0:q0 + 4, 0:H], op=OP.add)
                        eng = nc.sync if st % 2 == 0 else nc.scalar
                        eng.dma_start(out=y_r[:, q0:q0 + 4, :], in_=yout)

    nc.compile()
    return nc


def _get_nc():
    if "nc" not in _CACHE:
        _CACHE["nc"] = _build()
    return _CACHE["nc"]


def _in_maps(np_inputs):
    import ml_dtypes
    x = np.asarray(np_inputs["x"], dtype=np.float32)
    w = {}
    for k in ("Wa", "Wb", "Wv", "Wmlp", "ba", "bb", "bv", "bmlp", "Wc", "bc"):
        w[k] = np.ascontiguousarray(np.asarray(np_inputs[k], np.float32))
    in_maps = []
    for c in range(NCORES):
        b, h = divmod(c, 2)
        m = dict(w)
        # full batch of keys, rolled so this core's queries are rows 0:NQ
        m["xk"] = np.ascontiguousarray(
            np.concatenate([x[b, h * NQ:], x[b, :h * NQ]],
                           axis=0).astype(ml_dtypes.bfloat16))
        in_maps.append(m)
    return in_maps


def kernel(x, Wa, ba, Wb, bb, Wv, bv, Wc, bc, Wmlp, bmlp):
    from concourse.bass_utils import run_bass_kernel_spmd

    nc = _get_nc()
    in_maps = _in_maps(dict(x=x, Wa=Wa, ba=ba, Wb=Wb, bb=bb, Wv=Wv, bv=bv,
                            Wc=Wc, bc=bc, Wmlp=Wmlp, bmlp=bmlp))
    res = run_bass_kernel_spmd(nc, in_maps, core_ids=list(range(NCORES)))
    out = np.empty((B, N, H), np.float32)
    for c in range(NCORES):
        b, h = divmod(c, 2)
        out[b, h * NQ:(h + 1) * NQ] = res.results[c]["y"]
    return out


# revision 67
# speedup vs baseline: 1.0244x; 1.0088x over previous
"""Trainium2 Bass kernel for the GAT-style attention nn.Module.

Math: scores[b,i,j] = leaky_relu(sa_i + sb_j + bc) with sa = x@(Wa.T@wc_a)+ba.wc_a,
sb = x@(Wb.T@wc_b)+bb.wc_b.  Since exp(lrelu(t)) factorizes on each side of t=0
(exp(t)=E p_i q_j, exp(.01t)=E' p'_i q'_j) the softmax-weighted sum over keys
reduces to two masked sums over keys split at sb_j >= theta_i.  We bucketize sb
into K=128 quantized buckets, aggregate per-bucket sums of q*x (and q'*x) via a
one-hot matmul, project through Wv once per bucket, and resolve each query's
threshold with comparison-mask matmuls against the bucket tables.  Leaky-relu
continuity makes bucket-boundary misclassification error O(bucket width), so the
quantized split is numerically safe.  O(N*H + N*K*H/32) work instead of O(N^2*H).

Sharding: core c handles batch b=c//2, query half h=c%2.  Each core receives the
FULL batch's 4096 keys (host rolls x[b] so its 2048 queries are rows 0:2048) and
computes the bucket tables locally - no cross-core collective, so each core's
NEFF executes independently of the others' launch times.

x is shipped as bf16 (host-side dtype prep; halves the input DMA).  The score
dot products run on the DVE as x lands (every transpose route - XBAR DMA or PE
- costs ~11us/MB in descriptor or copy overhead, while the DVE starts at t~5us
with no staging).  Query-side sa runs first so the mask chain overlaps the
key-side sb dots; the one-hot -> bucket-sum matmuls interleave per 8-chunk
group.  Bucket counts ride a ones column in x.  All four strips' softmax
denominators come off the PE in one row; 1/den = exp(-ln(den)) on the ACT
tables, broadcast once, applied during the PSUM->SBUF copy of the numerators.
"""

import numpy as np

B, N, H = 4, 4096, 256
P = 128
NKCH = 32       # key chunks per core (full batch replicated)
QCH = 16        # query chunks
NQ = QCH * P    # 2048 queries per core
K = 128         # score buckets
NCORES = 8
NSTRIP = 4      # query strips of 512 for the lookup/mlp phase

_CACHE = {}


def _build(loop_n=None):
    import concourse.bacc as bacc
    import concourse.mybir as mybir
    from concourse.tile import TileContext
    from concourse.masks import make_identity

    F32 = mybir.dt.float32
    BF16 = mybir.dt.bfloat16
    I32 = mybir.dt.int32
    AF = mybir.ActivationFunctionType
    OP = mybir.AluOpType

    nc = bacc.Bacc("TRN2", target_bir_lowering=False, debug=False,
                   enable_asserts=False, num_devices=NCORES)

    xk_d = nc.dram_tensor("xk", [N, H], BF16, kind="ExternalInput")
    Wa_d = nc.dram_tensor("Wa", [H, H], F32, kind="ExternalInput")
    Wb_d = nc.dram_tensor("Wb", [H, H], F32, kind="ExternalInput")
    Wv_d = nc.dram_tensor("Wv", [H, H], F32, kind="ExternalInput")
    Wm_d = nc.dram_tensor("Wmlp", [H, H], F32, kind="ExternalInput")
    ba_d = nc.dram_tensor("ba", [H], F32, kind="ExternalInput")
    bb_d = nc.dram_tensor("bb", [H], F32, kind="ExternalInput")
    bv_d = nc.dram_tensor("bv", [H], F32, kind="ExternalInput")
    bm_d = nc.dram_tensor("bmlp", [H], F32, kind="ExternalInput")
    Wc_d = nc.dram_tensor("Wc", [1, 2 * H], F32, kind="ExternalInput")
    bc_d = nc.dram_tensor("bc", [1], F32, kind="ExternalInput")
    y_d = nc.dram_tensor("y", [NQ, H], F32, kind="ExternalOutput")

    xk_r = xk_d.ap().rearrange("(c p) f -> p c f", p=P)   # [128, 32, 256]
    y_r = y_d.ap().rearrange("(c p) f -> p c f", p=P)     # [128, 16, 256]

    with TileContext(nc) as tc:
        with tc.tile_pool(name="persist", bufs=1) as pp:

            import contextlib
            _loop = tc.For_i(0, loop_n, 1) if loop_n else contextlib.nullcontext()
            with _loop:
                # ---------- x row layout: query chunks on sync, keys after weights ----------
                xkb = pp.tile([P, NKCH, H + 1], BF16)
                nc.vector.memset(xkb[:, :, H:H + 1], 1.0)

                # wa/wca/wb/wcb gate the ua|ub row: they go FIRST, split over
                # both queues, then the x chunks, then the remaining weights
                wa_sb = pp.tile([P, 2, H], F32)
                wb_sb = pp.tile([P, 2, H], F32)
                wv_sb = pp.tile([P, 2, H], F32)
                wm_sb = pp.tile([P, 2, H], F32)
                wca = pp.tile([P, 2], F32)
                wcb = pp.tile([P, 2], F32)
                ba_c = pp.tile([P, 2], F32)
                bb_c = pp.tile([P, 2], F32)
                bm_c = pp.tile([P, 2], F32)
                bv_row = pp.tile([1, H], F32)
                bc_t = pp.tile([1, 1], F32)
                nc.sync.dma_start(out=wa_sb, in_=Wa_d.ap().rearrange("(c p) f -> p c f", p=P))
                nc.sync.dma_start(out=wca, in_=Wc_d.ap()[0:1, 0:H].rearrange("o (c p) -> p (o c)", p=P))
                nc.scalar.dma_start(out=wb_sb, in_=Wb_d.ap().rearrange("(c p) f -> p c f", p=P))
                nc.scalar.dma_start(out=wcb, in_=Wc_d.ap()[0:1, H:2 * H].rearrange("o (c p) -> p (o c)", p=P))
                for g in range(2):
                    nc.sync.dma_start(out=xkb[:, 8 * g:8 * g + 8, 0:H],
                                      in_=xk_r[:, 8 * g:8 * g + 8, :])
                    nc.scalar.dma_start(out=xkb[:, 16 + 8 * g:16 + 8 * g + 8, 0:H],
                                        in_=xk_r[:, 16 + 8 * g:16 + 8 * g + 8, :])
                nc.scalar.dma_start(out=ba_c, in_=ba_d.ap().rearrange("(c p) -> p c", p=P))
                nc.scalar.dma_start(out=bb_c, in_=bb_d.ap().rearrange("(c p) -> p c", p=P))
                nc.scalar.dma_start(out=bc_t, in_=bc_d.ap().rearrange("(o f) -> o f", o=1))
                nc.sync.dma_start(out=wv_sb, in_=Wv_d.ap().rearrange("(c p) f -> p c f", p=P))
                nc.sync.dma_start(out=wm_sb, in_=Wm_d.ap().rearrange("(c p) f -> p c f", p=P))
                nc.scalar.dma_start(out=bm_c, in_=bm_d.ap().rearrange("(c p) -> p c", p=P))
                nc.scalar.dma_start(out=bv_row, in_=bv_d.ap().rearrange("(o f) -> o f", o=1))

                # ---------- constants ----------
                iotac = pp.tile([P, 1], F32)            # value = partition idx
                nc.gpsimd.iota(iotac[:], pattern=[[0, 1]], base=0,
                               channel_multiplier=1,
                               allow_small_or_imprecise_dtypes=True)
                iota_b = pp.tile([P, K], BF16)          # value = bucket idx
                nc.gpsimd.iota(iota_b[:], pattern=[[1, K]], base=0,
                               channel_multiplier=0,
                               allow_small_or_imprecise_dtypes=True)
                identf = pp.tile([P, P], F32)
                identb = pp.tile([P, P], BF16)
                make_identity(nc, identf[:])

                # ---------- init compute: ua/ub rows, scalars, wvT/wmT ----------
                uab_rowb = pp.tile([1, 2 * H], BF16)
                sc3_row = pp.tile([1, 3], F32)       # (ca, cb, bc)
                wvT = pp.tile([P, 2, H], F32)    # Wv.T: [f_in, f_out]
                wmT = pp.tile([P, 2, H], BF16)   # Wmlp.T
                with tc.tile_pool(name="ps_u", bufs=1, space="PSUM") as ps_u, \
                     tc.tile_pool(name="ps_wt", bufs=1, space="PSUM") as ps_wt:
                    psu = ps_u.tile([1, 2 * H], F32, tag="psu")
                    for c in range(2):
                        nc.tensor.matmul(psu[0:1, 0:H], wca[:, c:c + 1], wa_sb[:, c, :],
                                         start=(c == 0), stop=(c == 1))
                    for c in range(2):
                        nc.tensor.matmul(psu[0:1, H:2 * H], wcb[:, c:c + 1], wb_sb[:, c, :],
                                         start=(c == 0), stop=(c == 1))
                    nc.vector.tensor_copy(out=uab_rowb, in_=psu)
                    psc = ps_u.tile([1, 2], F32, tag="psc")
                    for c in range(2):
                        nc.tensor.matmul(psc[0:1, 0:1], wca[:, c:c + 1], ba_c[:, c:c + 1],
                                         start=(c == 0), stop=(c == 1))
                    for c in range(2):
                        nc.tensor.matmul(psc[0:1, 1:2], wcb[:, c:c + 1], bb_c[:, c:c + 1],
                                         start=(c == 0), stop=(c == 1))
                    nc.vector.tensor_copy(out=sc3_row[0:1, 0:2], in_=psc)
                    nc.vector.tensor_copy(out=sc3_row[0:1, 2:3], in_=bc_t)

                    # weight transposes
                    for i in range(2):
                        for j in range(2):
                            pt = ps_wt.tile([P, P], F32, tag="wt")
                            nc.tensor.transpose(pt, wv_sb[:, i, j * P:(j + 1) * P], identf)
                            nc.scalar.copy(wvT[:, j, i * P:(i + 1) * P], pt)
                            pt2 = ps_wt.tile([P, P], F32, tag="wt2")
                            nc.tensor.transpose(pt2, wm_sb[:, i, j * P:(j + 1) * P], identf)
                            nc.vector.tensor_copy(out=wmT[:, j, i * P:(i + 1) * P], in_=pt2)

                # broadcasts (row already cast to bf16 by the psum copy)
                uab_b16 = pp.tile([P, 2 * H], BF16)
                nc.gpsimd.partition_broadcast(uab_b16[:], uab_rowb[:], channels=P)
                sc3 = pp.tile([P, 3], F32)           # cols: ca, cb, bc
                nc.gpsimd.partition_broadcast(sc3[:], sc3_row[:], channels=P)
                bv_bc = pp.tile([P, H], F32)
                nc.gpsimd.partition_broadcast(bv_bc[:], bv_row[:], channels=P)
                # identb built AFTER the broadcasts (gpsimd work, first used
                # by the strip-phase transposes emitted much later)
                make_identity(nc, identb[:])

                # ||ub||^2 as a per-partition self-dot of the broadcast row
                ubsq = pp.tile([P, 1], F32)
                with tc.tile_pool(name="scr0", bufs=2) as scr0:
                    d0 = scr0.tile([P, H], BF16, tag="d0")
                    nc.vector.scalar_tensor_tensor(
                        out=d0, in0=uab_b16[:, H:2 * H], scalar=0.0,
                        in1=uab_b16[:, H:2 * H], op0=OP.bypass, op1=OP.mult,
                        accum_out=ubsq)

                # ---------- query-side sa dots (DVE) ----------
                sah = pp.tile([P, QCH], F32)
                sbh = pp.tile([P, NKCH], F32)
                c_all = pp.tile([P, NKCH, K], BF16)
                with tc.tile_pool(name="scr", bufs=3) as scr:
                    for qc in range(QCH):
                        dsc = scr.tile([P, H], BF16, tag="dsc")
                        nc.vector.scalar_tensor_tensor(
                            out=dsc, in0=xkb[:, qc, 0:H], scalar=0.0,
                            in1=uab_b16[:, 0:H], op0=OP.bypass, op1=OP.mult,
                            accum_out=sah[:, qc:qc + 1])

                    # quantizer scalars (sig is already per-partition)
                    capbc = pp.tile([P, 1], F32)         # ca + bc
                    nc.vector.tensor_tensor(out=capbc, in0=sc3[:, 0:1],
                                            in1=sc3[:, 2:3], op=OP.add)
                    bias_pp = pp.tile([P, 1], F32)       # 0.01*(ca+bc)
                    nc.vector.tensor_scalar_mul(bias_pp, capbc, 0.01)
                    sig_col = pp.tile([P, 1], F32)
                    nc.scalar.activation(sig_col, ubsq, AF.Sqrt, bias=0.0, scale=1.0)
                    sig6 = pp.tile([P, 1], F32)          # 6.2 sigma
                    nc.vector.tensor_scalar_mul(sig6, sig_col, 6.2)
                    denom = pp.tile([P, 1], F32)         # full range = 12.4 sigma
                    nc.vector.tensor_scalar_mul(denom, sig_col, 12.4)
                    inv = pp.tile([P, 1], F32)
                    nc.vector.reciprocal(inv, denom)
                    scl = pp.tile([P, 1], F32)
                    nc.vector.tensor_scalar_mul(scl, inv, float(K))
                    nscl = pp.tile([P, 1], F32)
                    nc.vector.tensor_scalar_mul(nscl, scl, -1.0)
                    s1c = pp.tile([P, 1], F32)           # cb - lo_full = sig6
                    nc.vector.tensor_copy(out=s1c, in_=sig6)
                    lo_full = pp.tile([P, 1], F32)       # cb - sig6
                    nc.vector.tensor_tensor(out=lo_full, in0=sc3[:, 1:2], in1=sig6,
                                            op=OP.subtract)
                    s1d = pp.tile([P, 1], F32)           # ca + bc + lo_full
                    nc.vector.tensor_tensor(out=s1d, in0=capbc, in1=lo_full, op=OP.add)
                    w_col = pp.tile([P, 1], F32)
                    nc.vector.tensor_scalar_mul(w_col, denom, 1.0 / float(K))
                    ebias = pp.tile([P, 1], F32)     # lo_full + 0.5*w
                    nc.vector.tensor_scalar(out=ebias, in0=w_col, scalar1=0.5,
                                            scalar2=None, op0=OP.mult)
                    nc.vector.tensor_tensor(out=ebias, in0=ebias, in1=lo_full, op=OP.add)
                    e1_col = pp.tile([P, 1], F32)
                    e2_col = pp.tile([P, 1], F32)
                    ebias2 = pp.tile([P, 1], F32)
                    w2_col = pp.tile([P, 1], F32)
                    nc.vector.tensor_scalar_mul(ebias2, ebias, 0.01)
                    nc.vector.tensor_scalar_mul(w2_col, w_col, 0.01)
                    nc.scalar.activation(e1_col, iotac, AF.Exp, bias=ebias[:, 0:1],
                                         scale=w_col[:, 0:1])
                    nc.scalar.activation(e2_col, iotac, AF.Exp, bias=ebias2[:, 0:1],
                                         scale=w2_col[:, 0:1])
                    phat = pp.tile([P, QCH], F32)
                    phatp = pp.tile([P, QCH], F32)
                    nc.scalar.activation(phat, sah, AF.Exp, bias=capbc[:, 0:1], scale=1.0)
                    nc.scalar.activation(phatp, sah, AF.Exp, bias=bias_pp[:, 0:1],
                                         scale=0.01)

                    # query bucket index
                    d_f = pp.tile([P, QCH], F32)
                    d_i = pp.tile([P, QCH], I32)
                    nc.vector.tensor_scalar(out=d_f, in0=sah, scalar1=s1d[:, 0:1],
                                            scalar2=nscl[:, 0:1], op0=OP.add, op1=OP.mult)
                    nc.vector.tensor_scalar(out=d_f, in0=d_f, scalar1=-1.0,
                                            scalar2=float(K + 1), op0=OP.max, op1=OP.min)
                    nc.vector.tensor_copy(out=d_i, in_=d_f)
                    nc.vector.tensor_copy(out=d_f, in_=d_i)

                    # ---------- key-side sb dots + bucketize, per 8-chunk group ----------
                    c_f = pp.tile([P, NKCH], F32)
                    c_i = pp.tile([P, NKCH], I32)
                    for g in range(4):
                        for ci in range(8 * g, 8 * g + 8):
                            dsc = scr.tile([P, H], BF16, tag="dsc")
                            nc.vector.scalar_tensor_tensor(
                                out=dsc, in0=xkb[:, ci, 0:H], scalar=0.0,
                                in1=uab_b16[:, H:2 * H], op0=OP.bypass, op1=OP.mult,
                                accum_out=sbh[:, ci:ci + 1])
                        gs = slice(8 * g, 8 * g + 8)
                        nc.vector.tensor_scalar(out=c_f[:, gs], in0=sbh[:, gs],
                                                scalar1=s1c[:, 0:1], scalar2=scl[:, 0:1],
                                                op0=OP.add, op1=OP.mult)
                        nc.vector.tensor_scalar(out=c_f[:, gs], in0=c_f[:, gs],
                                                scalar1=0.0, scalar2=float(K - 1),
                                                op0=OP.max, op1=OP.min)
                        nc.vector.tensor_copy(out=c_i[:, gs], in_=c_f[:, gs])
                        nc.vector.tensor_copy(out=c_f[:, gs], in_=c_i[:, gs])
                        nc.vector.tensor_tensor(
                            out=c_all[:, gs, :],
                            in0=iota_b.unsqueeze(1).broadcast_to([P, 8, K]),
                            in1=c_f[:, gs].unsqueeze(2).broadcast_to([P, 8, K]),
                            op=OP.is_equal)

                # query masks fused with phat scaling (batched TTs)
                mge_p = pp.tile([P, QCH, K], BF16)
                mlt_p = pp.tile([P, QCH, K], BF16)
                iota_q = iota_b.unsqueeze(1).broadcast_to([P, QCH, K])
                nc.vector.tensor_tensor(
                    out=mge_p, in0=iota_q,
                    in1=d_f.unsqueeze(2).broadcast_to([P, QCH, K]), op=OP.is_ge)
                nc.vector.tensor_tensor(
                    out=mge_p, in0=mge_p,
                    in1=phat.unsqueeze(2).broadcast_to([P, QCH, K]), op=OP.mult)
                nc.vector.tensor_tensor(
                    out=mlt_p, in0=iota_q,
                    in1=d_f.unsqueeze(2).broadcast_to([P, QCH, K]), op=OP.is_lt)
                nc.vector.tensor_tensor(
                    out=mlt_p, in0=mlt_p,
                    in1=phatp.unsqueeze(2).broadcast_to([P, QCH, K]), op=OP.mult)
                fgeT = pp.tile([P, QCH, P], BF16)
                fltT = pp.tile([P, QCH, P], BF16)
                nc.sync.dma_start_transpose(out=fgeT[:], in_=mge_p[:])
                nc.scalar.dma_start_transpose(out=fltT[:], in_=mlt_p[:])

                # ---------- one-hot + bucket aggregation, interleaved per group ----------
                tabS = pp.tile([P, H], BF16)
                tabT = pp.tile([P, H], BF16)
                g1s = pp.tile([P, H + 1], F32)
                g2s = pp.tile([P, H + 1], F32)
                gq_colb = pp.tile([P, 2], BF16)        # e-scaled bucket counts
                rln = pp.tile([1, NSTRIP * 4 * P], F32)
                r_row = pp.tile([1, NSTRIP * 4 * P], F32)
                rbc = pp.tile([P, NSTRIP * 4 * P], F32)
                with tc.tile_pool(name="ps_g", bufs=1, space="PSUM") as ps_g, \
                     tc.tile_pool(name="ps_den", bufs=1, space="PSUM") as ps_den, \
                     tc.tile_pool(name="ps_t2", bufs=1, space="PSUM") as ps_t2, \
                     tc.tile_pool(name="ps_gv", bufs=1, space="PSUM") as ps_gv:
                    G1 = ps_g.tile([P, H + 1], F32, tag="G1")
                    for ci in range(NKCH):
                        nc.tensor.matmul(G1, c_all[:, ci, :], xkb[:, ci, :],
                                         start=(ci == 0), stop=(ci == NKCH - 1))
                    # q ~ const per bucket: row-scale raw sums by e1/e2
                    nc.vector.tensor_scalar(out=g1s, in0=G1, scalar1=e1_col[:, 0:1],
                                            scalar2=None, op0=OP.mult)
                    nc.vector.tensor_scalar(out=g2s, in0=G1, scalar1=e2_col[:, 0:1],
                                            scalar2=None, op0=OP.mult)
                    nc.vector.tensor_copy(out=gq_colb[:, 0:1], in_=g1s[:, H:H + 1])
                    nc.vector.tensor_copy(out=gq_colb[:, 1:2], in_=g2s[:, H:H + 1])

                    # all-strip denominators: one [1, 2048] row off the PE
                    pden = ps_den.tile([1, NSTRIP * 4 * P], F32, tag="pden")
                    for st in range(NSTRIP):
                        q0 = 4 * st
                        sl = slice(512 * st, 512 * (st + 1))
                        nc.tensor.matmul(pden[0:1, sl], gq_colb[:, 0:1],
                                         fgeT[:, q0:q0 + 4, :], start=True, stop=False)
                        nc.tensor.matmul(pden[0:1, sl], gq_colb[:, 1:2],
                                         fltT[:, q0:q0 + 4, :], start=False, stop=True)
                    # 1/den = exp(-ln(den)) on the ACT tables (the DVE
                    # reciprocal is ~6.5ns/elem and would serialize strips);
                    # broadcast per strip so attnT(0) starts sooner
                    nc.scalar.activation(rln, pden, AF.Ln, bias=0.0, scale=1.0)
                    nc.scalar.activation(r_row, rln, AF.Exp, bias=0.0, scale=-1.0)
                    for st in range(NSTRIP):
                        sl = slice(512 * st, 512 * (st + 1))
                        nc.gpsimd.partition_broadcast(rbc[:, sl], r_row[0:1, sl],
                                                      channels=P)

                    # transpose Gx and project through Wv.T
                    gxT1 = pp.tile([P, 2, K], F32)
                    gxT2 = pp.tile([P, 2, K], F32)
                    for j in range(2):
                        pt = ps_t2.tile([P, P], F32, tag="tp")
                        nc.tensor.transpose(pt, g1s[:, j * P:(j + 1) * P], identf)
                        nc.scalar.copy(gxT1[:, j, :], pt)
                        pt2 = ps_t2.tile([P, P], F32, tag="tp")
                        nc.tensor.transpose(pt2, g2s[:, j * P:(j + 1) * P], identf)
                        nc.scalar.copy(gxT2[:, j, :], pt2)
                    Gv1 = ps_gv.tile([P, H], F32, tag="Gv1")
                    Gv2 = ps_gv.tile([P, H], F32, tag="Gv2")
                    for j in range(2):
                        nc.tensor.matmul(Gv1, gxT1[:, j, :], wvT[:, j, :],
                                         start=(j == 0), stop=(j == 1))
                    for j in range(2):
                        nc.tensor.matmul(Gv2, gxT2[:, j, :], wvT[:, j, :],
                                         start=(j == 0), stop=(j == 1))
                    # tab = Gv + gq * bv   (outer product via per-partition scalar)
                    nc.vector.scalar_tensor_tensor(out=tabS, in0=bv_bc,
                                                   scalar=g1s[:, H:H + 1], in1=Gv1,
                                                   op0=OP.mult, op1=OP.add)
                    nc.vector.scalar_tensor_tensor(out=tabT, in0=bv_bc,
                                                   scalar=g2s[:, H:H + 1], in1=Gv2,
                                                   op0=OP.mult, op1=OP.add)

                # ---------- query tail, software-pipelined strips of 512 ----------
                def _pnum(ps_num, st):
                    q0 = 4 * st
                    pn = ps_num.tile([P, 2, 512], F32, tag="pnum")
                    for m in range(2):
                        nc.tensor.matmul(pn[:, m, :], tabS[:, m * P:(m + 1) * P],
                                         fgeT[:, q0:q0 + 4, :], start=True, stop=False)
                        nc.tensor.matmul(pn[:, m, :], tabT[:, m * P:(m + 1) * P],
                                         fltT[:, q0:q0 + 4, :], start=False, stop=True)
                    return pn

                with tc.tile_pool(name="ps_num", bufs=3, space="PSUM") as ps_num, \
                     tc.tile_pool(name="ps_y", bufs=2, space="PSUM") as ps_y, \
                     tc.tile_pool(name="strip", bufs=3) as sp:
                    pnum = _pnum(ps_num, 0)
                    for st in range(NSTRIP):
                        q0 = 4 * st
                        # attn = num / den, fused into the PSUM->SBUF copy
                        attnT = sp.tile([P, 2, 512], BF16, tag="attnT")
                        nc.vector.scalar_tensor_tensor(
                            out=attnT, in0=pnum, scalar=0.0,
                            in1=rbc[:, 512 * st:512 * (st + 1)]
                            .unsqueeze(1).broadcast_to([P, 2, 512]),
                            op0=OP.bypass, op1=OP.mult)
                        # keep the PE streaming: next strip's pnum before pz
                        if st + 1 < NSTRIP:
                            pnum = _pnum(ps_num, st + 1)

                        pz = ps_num.tile([P, 2, 512], F32, tag="pnum")
                        for mo in range(2):
                            nc.tensor.matmul(pz[:, mo, :],
                                             wmT[:, 0, mo * P:(mo + 1) * P],
                                             attnT[:, 0, :], start=True, stop=False)
                            nc.tensor.matmul(pz[:, mo, :],
                                             wmT[:, 1, mo * P:(mo + 1) * P],
                                             attnT[:, 1, :], start=False, stop=True)
                        yt = sp.tile([P, 2, 512], BF16, tag="yt")
                        for mo in range(2):
                            nc.scalar.activation(yt[:, mo, :], pz[:, mo, :], AF.Tanh,
                                                 bias=bm_c[:, mo:mo + 1], scale=1.0)

                        # transpose y back to query-partition layout on the PE
                        py = ps_y.tile([P, 4, H], BF16, tag="py")
                        for qq in range(4):
                            for fc in range(2):
                                nc.tensor.transpose(py[:, qq, fc * P:(fc + 1) * P],
                                                    yt[:, fc, qq * P:(qq + 1) * P],
                                                    identb)
                        yout = sp.tile([P, 4, H], F32, tag="yout")
                        nc.vector.tensor_tensor(out=yout, in0=py,
                                                in1=xkb[:, q0:q0 + 4, 0:H], op=OP.add)
                        eng = nc.sync if st % 2 == 0 else nc.scalar
                        eng.dma_start(out=y_r[:, q0:q0 + 4, :], in_=yout)

    nc.compile()
    return nc


def _get_nc():
    if "nc" not in _CACHE:
        _CACHE["nc"] = _build()
    return _CACHE["nc"]


def _in_maps(np_inputs):
    import ml_dtypes
    x = np.asarray(np_inputs["x"], dtype=np.float32)
    w = {}
    for k in ("Wa", "Wb", "Wv", "Wmlp", "ba", "bb", "bv", "bmlp", "Wc", "bc"):
        w[k] = np.ascontiguousarray(np.asarray(np_inputs[k], np.float32))
    in_maps = []
    for c in range(NCORES):
        b, h = divmod(c, 2)
        m = dict(w)
        # full batch of keys, rolled so this core's queries are rows 0:NQ
        m["xk"] = np.ascontiguousarray(
            np.concatenate([x[b, h * NQ:], x[b, :h * NQ]],
                           axis=0).astype(ml_dtypes.bfloat16))
        in_maps.append(m)
    return in_maps


def kernel(x, Wa, ba, Wb, bb, Wv, bv, Wc, bc, Wmlp, bmlp):
    from concourse.bass_utils import run_bass_kernel_spmd

    nc = _get_nc()
    in_maps = _in_maps(dict(x=x, Wa=Wa, ba=ba, Wb=Wb, bb=bb, Wv=Wv, bv=bv,
                            Wc=Wc, bc=bc, Wmlp=Wmlp, bmlp=bmlp))
    res = run_bass_kernel_spmd(nc, in_maps, core_ids=list(range(NCORES)))
    out = np.empty((B, N, H), np.float32)
    for c in range(NCORES):
        b, h = divmod(c, 2)
        out[b, h * NQ:(h + 1) * NQ] = res.results[c]["y"]
    return out


# revision 68
# speedup vs baseline: 1.0468x; 1.0219x over previous
"""Trainium2 Bass kernel for the GAT-style attention nn.Module.

Math: scores[b,i,j] = leaky_relu(sa_i + sb_j + bc) with sa = x@(Wa.T@wc_a)+ba.wc_a,
sb = x@(Wb.T@wc_b)+bb.wc_b.  Since exp(lrelu(t)) factorizes on each side of t=0
(exp(t)=E p_i q_j, exp(.01t)=E' p'_i q'_j) the softmax-weighted sum over keys
reduces to two masked sums over keys split at sb_j >= theta_i.  We bucketize sb
into K=128 quantized buckets, aggregate per-bucket sums of q*x (and q'*x) via a
one-hot matmul, project through Wv once per bucket, and resolve each query's
threshold with comparison-mask matmuls against the bucket tables.  Leaky-relu
continuity makes bucket-boundary misclassification error O(bucket width), so the
quantized split is numerically safe.  O(N*H + N*K*H/32) work instead of O(N^2*H).

Sharding: core c handles batch b=c//2, query half h=c%2.  Each core receives the
FULL batch's 4096 keys (host rolls x[b] so its 2048 queries are rows 0:2048) and
computes the bucket tables locally - no cross-core collective, so each core's
NEFF executes independently of the others' launch times.

x is shipped as bf16 (host-side dtype prep; halves the input DMA).  The score
dot products run on the DVE as x lands (every transpose route - XBAR DMA or PE
- costs ~11us/MB in descriptor or copy overhead, while the DVE starts at t~5us
with no staging).  Query-side sa runs first so the mask chain overlaps the
key-side sb dots; the one-hot -> bucket-sum matmuls interleave per 8-chunk
group.  Bucket counts ride a ones column in x.  All four strips' softmax
denominators come off the PE in one row; 1/den = exp(-ln(den)) on the ACT
tables, broadcast once, applied during the PSUM->SBUF copy of the numerators.
"""

import numpy as np

B, N, H = 4, 4096, 256
P = 128
NKCH = 32       # key chunks per core (full batch replicated)
QCH = 16        # query chunks
NQ = QCH * P    # 2048 queries per core
K = 128         # score buckets
NCORES = 8
NSTRIP = 4      # query strips of 512 for the lookup/mlp phase

_CACHE = {}


def _build(loop_n=None):
    import concourse.bacc as bacc
    import concourse.mybir as mybir
    from concourse.tile import TileContext
    from concourse.masks import make_identity

    F32 = mybir.dt.float32
    BF16 = mybir.dt.bfloat16
    I32 = mybir.dt.int32
    AF = mybir.ActivationFunctionType
    OP = mybir.AluOpType

    nc = bacc.Bacc("TRN2", target_bir_lowering=False, debug=False,
                   enable_asserts=False, num_devices=NCORES)

    xk_d = nc.dram_tensor("xk", [N, H], BF16, kind="ExternalInput")
    Wa_d = nc.dram_tensor("Wa", [H, H], F32, kind="ExternalInput")
    Wb_d = nc.dram_tensor("Wb", [H, H], F32, kind="ExternalInput")
    Wv_d = nc.dram_tensor("Wv", [H, H], F32, kind="ExternalInput")
    Wm_d = nc.dram_tensor("Wmlp", [H, H], F32, kind="ExternalInput")
    ba_d = nc.dram_tensor("ba", [H], F32, kind="ExternalInput")
    bb_d = nc.dram_tensor("bb", [H], F32, kind="ExternalInput")
    bv_d = nc.dram_tensor("bv", [H], F32, kind="ExternalInput")
    bm_d = nc.dram_tensor("bmlp", [H], F32, kind="ExternalInput")
    Wc_d = nc.dram_tensor("Wc", [1, 2 * H], F32, kind="ExternalInput")
    bc_d = nc.dram_tensor("bc", [1], F32, kind="ExternalInput")
    y_d = nc.dram_tensor("y", [NQ, H], F32, kind="ExternalOutput")

    xk_r = xk_d.ap().rearrange("(c p) f -> p c f", p=P)   # [128, 32, 256]
    y_r = y_d.ap().rearrange("(c p) f -> p c f", p=P)     # [128, 16, 256]

    with TileContext(nc) as tc:
        with tc.tile_pool(name="persist", bufs=1) as pp:

            import contextlib
            _loop = tc.For_i(0, loop_n, 1) if loop_n else contextlib.nullcontext()
            with _loop:
                # ---------- x row layout: query chunks on sync, keys after weights ----------
                xkb = pp.tile([P, NKCH, H + 1], BF16)
                nc.vector.memset(xkb[:, :, H:H + 1], 1.0)

                # wa/wca/wb/wcb gate the ua|ub row: they go FIRST, split over
                # both queues, then the x chunks, then the remaining weights
                wa_sb = pp.tile([P, 2, H], F32)
                wb_sb = pp.tile([P, 2, H], F32)
                wv_sb = pp.tile([P, 2, H], F32)
                wm_sb = pp.tile([P, 2, H], F32)
                wca = pp.tile([P, 2], F32)
                wcb = pp.tile([P, 2], F32)
                ba_c = pp.tile([P, 2], F32)
                bb_c = pp.tile([P, 2], F32)
                bm_c = pp.tile([P, 2], F32)
                bv_row = pp.tile([1, H], F32)
                bc_t = pp.tile([1, 1], F32)
                nc.sync.dma_start(out=wa_sb, in_=Wa_d.ap().rearrange("(c p) f -> p c f", p=P))
                nc.sync.dma_start(out=wca, in_=Wc_d.ap()[0:1, 0:H].rearrange("o (c p) -> p (o c)", p=P))
                nc.scalar.dma_start(out=wb_sb, in_=Wb_d.ap().rearrange("(c p) f -> p c f", p=P))
                nc.scalar.dma_start(out=wcb, in_=Wc_d.ap()[0:1, H:2 * H].rearrange("o (c p) -> p (o c)", p=P))
                for g in range(2):
                    nc.sync.dma_start(out=xkb[:, 8 * g:8 * g + 8, 0:H],
                                      in_=xk_r[:, 8 * g:8 * g + 8, :])
                    nc.scalar.dma_start(out=xkb[:, 16 + 8 * g:16 + 8 * g + 8, 0:H],
                                        in_=xk_r[:, 16 + 8 * g:16 + 8 * g + 8, :])
                nc.scalar.dma_start(out=ba_c, in_=ba_d.ap().rearrange("(c p) -> p c", p=P))
                nc.scalar.dma_start(out=bb_c, in_=bb_d.ap().rearrange("(c p) -> p c", p=P))
                nc.scalar.dma_start(out=bc_t, in_=bc_d.ap().rearrange("(o f) -> o f", o=1))
                nc.sync.dma_start(out=wv_sb, in_=Wv_d.ap().rearrange("(c p) f -> p c f", p=P))
                nc.sync.dma_start(out=wm_sb, in_=Wm_d.ap().rearrange("(c p) f -> p c f", p=P))
                nc.scalar.dma_start(out=bm_c, in_=bm_d.ap().rearrange("(c p) -> p c", p=P))
                nc.scalar.dma_start(out=bv_row, in_=bv_d.ap().rearrange("(o f) -> o f", o=1))

                # ---------- constants ----------
                iotac = pp.tile([P, 1], F32)            # value = partition idx
                nc.gpsimd.iota(iotac[:], pattern=[[0, 1]], base=0,
                               channel_multiplier=1,
                               allow_small_or_imprecise_dtypes=True)
                iota_b = pp.tile([P, K], BF16)          # value = bucket idx
                nc.gpsimd.iota(iota_b[:], pattern=[[1, K]], base=0,
                               channel_multiplier=0,
                               allow_small_or_imprecise_dtypes=True)
                identf = pp.tile([P, P], F32)
                identb = pp.tile([P, P], BF16)
                make_identity(nc, identf[:])

                # ---------- init compute: ua/ub rows, scalars, wvT/wmT ----------
                uab_rowb = pp.tile([1, 2 * H], BF16)
                sc3_row = pp.tile([1, 3], F32)       # (ca, cb, bc)
                wvT = pp.tile([P, 2, H], F32)    # Wv.T: [f_in, f_out]
                wmT = pp.tile([P, 2, H], BF16)   # Wmlp.T
                with tc.tile_pool(name="ps_u", bufs=1, space="PSUM") as ps_u, \
                     tc.tile_pool(name="ps_wt", bufs=2, space="PSUM") as ps_wt:
                    psu = ps_u.tile([1, 2 * H], F32, tag="psu")
                    for c in range(2):
                        nc.tensor.matmul(psu[0:1, 0:H], wca[:, c:c + 1], wa_sb[:, c, :],
                                         start=(c == 0), stop=(c == 1))
                    for c in range(2):
                        nc.tensor.matmul(psu[0:1, H:2 * H], wcb[:, c:c + 1], wb_sb[:, c, :],
                                         start=(c == 0), stop=(c == 1))
                    nc.vector.tensor_copy(out=uab_rowb, in_=psu)
                    psc = ps_u.tile([1, 2], F32, tag="psc")
                    for c in range(2):
                        nc.tensor.matmul(psc[0:1, 0:1], wca[:, c:c + 1], ba_c[:, c:c + 1],
                                         start=(c == 0), stop=(c == 1))
                    for c in range(2):
                        nc.tensor.matmul(psc[0:1, 1:2], wcb[:, c:c + 1], bb_c[:, c:c + 1],
                                         start=(c == 0), stop=(c == 1))
                    nc.vector.tensor_copy(out=sc3_row[0:1, 0:2], in_=psc)
                    nc.vector.tensor_copy(out=sc3_row[0:1, 2:3], in_=bc_t)

                    # weight transposes
                    for i in range(2):
                        for j in range(2):
                            pt = ps_wt.tile([P, P], F32, tag="wt")
                            nc.tensor.transpose(pt, wv_sb[:, i, j * P:(j + 1) * P], identf)
                            nc.scalar.copy(wvT[:, j, i * P:(i + 1) * P], pt)
                            pt2 = ps_wt.tile([P, P], F32, tag="wt2")
                            nc.tensor.transpose(pt2, wm_sb[:, i, j * P:(j + 1) * P], identf)
                            nc.vector.tensor_copy(out=wmT[:, j, i * P:(i + 1) * P], in_=pt2)

                # broadcasts (row already cast to bf16 by the psum copy)
                uab_b16 = pp.tile([P, 2 * H], BF16)
                nc.gpsimd.partition_broadcast(uab_b16[:], uab_rowb[:], channels=P)
                sc3 = pp.tile([P, 3], F32)           # cols: ca, cb, bc
                nc.gpsimd.partition_broadcast(sc3[:], sc3_row[:], channels=P)
                bv_bc = pp.tile([P, H], F32)
                nc.gpsimd.partition_broadcast(bv_bc[:], bv_row[:], channels=P)
                # identb built AFTER the broadcasts (gpsimd work, first used
                # by the strip-phase transposes emitted much later)
                make_identity(nc, identb[:])

                # ||ub||^2 as a per-partition self-dot of the broadcast row
                ubsq = pp.tile([P, 1], F32)
                with tc.tile_pool(name="scr0", bufs=2) as scr0:
                    d0 = scr0.tile([P, H], BF16, tag="d0")
                    nc.vector.scalar_tensor_tensor(
                        out=d0, in0=uab_b16[:, H:2 * H], scalar=0.0,
                        in1=uab_b16[:, H:2 * H], op0=OP.bypass, op1=OP.mult,
                        accum_out=ubsq)

                # ---------- query-side sa dots (DVE) ----------
                sah = pp.tile([P, QCH], F32)
                sbh = pp.tile([P, NKCH], F32)
                c_all = pp.tile([P, NKCH, K], BF16)
                with tc.tile_pool(name="scr", bufs=3) as scr:
                    for qc in range(QCH):
                        dsc = scr.tile([P, H], BF16, tag="dsc")
                        nc.vector.scalar_tensor_tensor(
                            out=dsc, in0=xkb[:, qc, 0:H], scalar=0.0,
                            in1=uab_b16[:, 0:H], op0=OP.bypass, op1=OP.mult,
                            accum_out=sah[:, qc:qc + 1])

                    # quantizer scalars (sig is already per-partition)
                    capbc = pp.tile([P, 1], F32)         # ca + bc
                    nc.vector.tensor_tensor(out=capbc, in0=sc3[:, 0:1],
                                            in1=sc3[:, 2:3], op=OP.add)
                    bias_pp = pp.tile([P, 1], F32)       # 0.01*(ca+bc)
                    nc.vector.tensor_scalar_mul(bias_pp, capbc, 0.01)
                    sig_col = pp.tile([P, 1], F32)
                    nc.scalar.activation(sig_col, ubsq, AF.Sqrt, bias=0.0, scale=1.0)
                    sig6 = pp.tile([P, 1], F32)          # 6.2 sigma
                    nc.vector.tensor_scalar_mul(sig6, sig_col, 6.2)
                    denom = pp.tile([P, 1], F32)         # full range = 12.4 sigma
                    nc.vector.tensor_scalar_mul(denom, sig_col, 12.4)
                    inv = pp.tile([P, 1], F32)
                    nc.vector.reciprocal(inv, denom)
                    scl = pp.tile([P, 1], F32)
                    nc.vector.tensor_scalar_mul(scl, inv, float(K))
                    nscl = pp.tile([P, 1], F32)
                    nc.vector.tensor_scalar_mul(nscl, scl, -1.0)
                    s1c = pp.tile([P, 1], F32)           # cb - lo_full = sig6
                    nc.vector.tensor_copy(out=s1c, in_=sig6)
                    lo_full = pp.tile([P, 1], F32)       # cb - sig6
                    nc.vector.tensor_tensor(out=lo_full, in0=sc3[:, 1:2], in1=sig6,
                                            op=OP.subtract)
                    s1d = pp.tile([P, 1], F32)           # ca + bc + lo_full
                    nc.vector.tensor_tensor(out=s1d, in0=capbc, in1=lo_full, op=OP.add)
                    w_col = pp.tile([P, 1], F32)
                    nc.vector.tensor_scalar_mul(w_col, denom, 1.0 / float(K))
                    ebias = pp.tile([P, 1], F32)     # lo_full + 0.5*w
                    nc.vector.tensor_scalar(out=ebias, in0=w_col, scalar1=0.5,
                                            scalar2=None, op0=OP.mult)
                    nc.vector.tensor_tensor(out=ebias, in0=ebias, in1=lo_full, op=OP.add)
                    e1_col = pp.tile([P, 1], F32)
                    e2_col = pp.tile([P, 1], F32)
                    ebias2 = pp.tile([P, 1], F32)
                    w2_col = pp.tile([P, 1], F32)
                    nc.vector.tensor_scalar_mul(ebias2, ebias, 0.01)
                    nc.vector.tensor_scalar_mul(w2_col, w_col, 0.01)
                    nc.scalar.activation(e1_col, iotac, AF.Exp, bias=ebias[:, 0:1],
                                         scale=w_col[:, 0:1])
                    nc.scalar.activation(e2_col, iotac, AF.Exp, bias=ebias2[:, 0:1],
                                         scale=w2_col[:, 0:1])
                    phat = pp.tile([P, QCH], F32)
                    phatp = pp.tile([P, QCH], F32)
                    nc.scalar.activation(phat, sah, AF.Exp, bias=capbc[:, 0:1], scale=1.0)
                    nc.scalar.activation(phatp, sah, AF.Exp, bias=bias_pp[:, 0:1],
                                         scale=0.01)

                    # query bucket index
                    d_f = pp.tile([P, QCH], F32)
                    d_i = pp.tile([P, QCH], I32)
                    nc.vector.tensor_scalar(out=d_f, in0=sah, scalar1=s1d[:, 0:1],
                                            scalar2=nscl[:, 0:1], op0=OP.add, op1=OP.mult)
                    nc.vector.tensor_scalar(out=d_f, in0=d_f, scalar1=-1.0,
                                            scalar2=float(K + 1), op0=OP.max, op1=OP.min)
                    nc.vector.tensor_copy(out=d_i, in_=d_f)
                    nc.vector.tensor_copy(out=d_f, in_=d_i)

                    # ---------- key-side sb dots + bucketize, per 8-chunk group ----------
                    c_f = pp.tile([P, NKCH], F32)
                    c_i = pp.tile([P, NKCH], I32)
                    for g in range(4):
                        for ci in range(8 * g, 8 * g + 8):
                            dsc = scr.tile([P, H], BF16, tag="dsc")
                            nc.vector.scalar_tensor_tensor(
                                out=dsc, in0=xkb[:, ci, 0:H], scalar=0.0,
                                in1=uab_b16[:, H:2 * H], op0=OP.bypass, op1=OP.mult,
                                accum_out=sbh[:, ci:ci + 1])
                        gs = slice(8 * g, 8 * g + 8)
                        nc.vector.tensor_scalar(out=c_f[:, gs], in0=sbh[:, gs],
                                                scalar1=s1c[:, 0:1], scalar2=scl[:, 0:1],
                                                op0=OP.add, op1=OP.mult)
                        nc.vector.tensor_scalar(out=c_f[:, gs], in0=c_f[:, gs],
                                                scalar1=0.0, scalar2=float(K - 1),
                                                op0=OP.max, op1=OP.min)
                        nc.vector.tensor_copy(out=c_i[:, gs], in_=c_f[:, gs])
                        nc.vector.tensor_copy(out=c_f[:, gs], in_=c_i[:, gs])
                        nc.vector.tensor_tensor(
                            out=c_all[:, gs, :],
                            in0=iota_b.unsqueeze(1).broadcast_to([P, 8, K]),
                            in1=c_f[:, gs].unsqueeze(2).broadcast_to([P, 8, K]),
                            op=OP.is_equal)

                # query masks fused with phat scaling (batched TTs)
                mge_p = pp.tile([P, QCH, K], BF16)
                mlt_p = pp.tile([P, QCH, K], BF16)
                iota_q = iota_b.unsqueeze(1).broadcast_to([P, QCH, K])
                nc.vector.tensor_tensor(
                    out=mge_p, in0=iota_q,
                    in1=d_f.unsqueeze(2).broadcast_to([P, QCH, K]), op=OP.is_ge)
                nc.vector.tensor_tensor(
                    out=mge_p, in0=mge_p,
                    in1=phat.unsqueeze(2).broadcast_to([P, QCH, K]), op=OP.mult)
                nc.vector.tensor_tensor(
                    out=mlt_p, in0=iota_q,
                    in1=d_f.unsqueeze(2).broadcast_to([P, QCH, K]), op=OP.is_lt)
                nc.vector.tensor_tensor(
                    out=mlt_p, in0=mlt_p,
                    in1=phatp.unsqueeze(2).broadcast_to([P, QCH, K]), op=OP.mult)
                fgeT = pp.tile([P, QCH, P], BF16)
                fltT = pp.tile([P, QCH, P], BF16)
                nc.sync.dma_start_transpose(out=fgeT[:], in_=mge_p[:])
                nc.scalar.dma_start_transpose(out=fltT[:], in_=mlt_p[:])

                # ---------- one-hot + bucket aggregation, interleaved per group ----------
                tabS = pp.tile([P, H], BF16)
                tabT = pp.tile([P, H], BF16)
                g1s = pp.tile([P, H + 1], F32)
                g2s = pp.tile([P, H + 1], F32)
                gq_colb = pp.tile([P, 2], BF16)        # e-scaled bucket counts
                rln = pp.tile([1, NSTRIP * 4 * P], F32)
                r_row = pp.tile([1, NSTRIP * 4 * P], F32)
                rbc = pp.tile([P, NSTRIP * 4 * P], F32)
                with tc.tile_pool(name="ps_g", bufs=1, space="PSUM") as ps_g, \
                     tc.tile_pool(name="ps_den", bufs=1, space="PSUM") as ps_den, \
                     tc.tile_pool(name="ps_t2", bufs=1, space="PSUM") as ps_t2, \
                     tc.tile_pool(name="ps_gv", bufs=1, space="PSUM") as ps_gv:
                    G1 = ps_g.tile([P, H + 1], F32, tag="G1")
                    for ci in range(NKCH):
                        nc.tensor.matmul(G1, c_all[:, ci, :], xkb[:, ci, :],
                                         start=(ci == 0), stop=(ci == NKCH - 1))
                    # q ~ const per bucket: row-scale raw sums by e1/e2
                    nc.vector.tensor_scalar(out=g1s, in0=G1, scalar1=e1_col[:, 0:1],
                                            scalar2=None, op0=OP.mult)
                    nc.vector.tensor_scalar(out=g2s, in0=G1, scalar1=e2_col[:, 0:1],
                                            scalar2=None, op0=OP.mult)
                    nc.vector.tensor_copy(out=gq_colb[:, 0:1], in_=g1s[:, H:H + 1])
                    nc.vector.tensor_copy(out=gq_colb[:, 1:2], in_=g2s[:, H:H + 1])

                    # all-strip denominators: one [1, 2048] row off the PE
                    pden = ps_den.tile([1, NSTRIP * 4 * P], F32, tag="pden")
                    for st in range(NSTRIP):
                        q0 = 4 * st
                        sl = slice(512 * st, 512 * (st + 1))
                        nc.tensor.matmul(pden[0:1, sl], gq_colb[:, 0:1],
                                         fgeT[:, q0:q0 + 4, :], start=True, stop=False)
                        nc.tensor.matmul(pden[0:1, sl], gq_colb[:, 1:2],
                                         fltT[:, q0:q0 + 4, :], start=False, stop=True)
                    # 1/den = exp(-ln(den)) on the ACT tables (the DVE
                    # reciprocal is ~6.5ns/elem and would serialize strips);
                    # broadcast per strip so attnT(0) starts sooner
                    nc.scalar.activation(rln, pden, AF.Ln, bias=0.0, scale=1.0)
                    nc.scalar.activation(r_row, rln, AF.Exp, bias=0.0, scale=-1.0)
                    for st in range(NSTRIP):
                        sl = slice(512 * st, 512 * (st + 1))
                        nc.gpsimd.partition_broadcast(rbc[:, sl], r_row[0:1, sl],
                                                      channels=P)

                    # transpose Gx and project through Wv.T
                    gxT1 = pp.tile([P, 2, K], F32)
                    gxT2 = pp.tile([P, 2, K], F32)
                    for j in range(2):
                        pt = ps_t2.tile([P, P], F32, tag="tp")
                        nc.tensor.transpose(pt, g1s[:, j * P:(j + 1) * P], identf)
                        nc.scalar.copy(gxT1[:, j, :], pt)
                        pt2 = ps_t2.tile([P, P], F32, tag="tp")
                        nc.tensor.transpose(pt2, g2s[:, j * P:(j + 1) * P], identf)
                        nc.scalar.copy(gxT2[:, j, :], pt2)
                    Gv1 = ps_gv.tile([P, H], F32, tag="Gv1")
                    Gv2 = ps_gv.tile([P, H], F32, tag="Gv2")
                    for j in range(2):
                        nc.tensor.matmul(Gv1, gxT1[:, j, :], wvT[:, j, :],
                                         start=(j == 0), stop=(j == 1))
                    for j in range(2):
                        nc.tensor.matmul(Gv2, gxT2[:, j, :], wvT[:, j, :],
                                         start=(j == 0), stop=(j == 1))
                    # tab = Gv + gq * bv   (outer product via per-partition scalar)
                    nc.vector.scalar_tensor_tensor(out=tabS, in0=bv_bc,
                                                   scalar=g1s[:, H:H + 1], in1=Gv1,
                                                   op0=OP.mult, op1=OP.add)
                    nc.vector.scalar_tensor_tensor(out=tabT, in0=bv_bc,
                                                   scalar=g2s[:, H:H + 1], in1=Gv2,
                                                   op0=OP.mult, op1=OP.add)

                # ---------- query tail, software-pipelined strips of 512 ----------
                def _pnum(ps_num, st):
                    q0 = 4 * st
                    pn = ps_num.tile([P, 2, 512], F32, tag="pnum")
                    for m in range(2):
                        nc.tensor.matmul(pn[:, m, :], tabS[:, m * P:(m + 1) * P],
                                         fgeT[:, q0:q0 + 4, :], start=True, stop=False)
                        nc.tensor.matmul(pn[:, m, :], tabT[:, m * P:(m + 1) * P],
                                         fltT[:, q0:q0 + 4, :], start=False, stop=True)
                    return pn

                with tc.tile_pool(name="ps_num", bufs=3, space="PSUM") as ps_num, \
                     tc.tile_pool(name="ps_y", bufs=2, space="PSUM") as ps_y, \
                     tc.tile_pool(name="strip", bufs=4) as sp:
                    pnum = _pnum(ps_num, 0)
                    for st in range(NSTRIP):
                        q0 = 4 * st
                        # attn = num / den, fused into the PSUM->SBUF copy
                        attnT = sp.tile([P, 2, 512], BF16, tag="attnT")
                        nc.vector.scalar_tensor_tensor(
                            out=attnT, in0=pnum, scalar=0.0,
                            in1=rbc[:, 512 * st:512 * (st + 1)]
                            .unsqueeze(1).broadcast_to([P, 2, 512]),
                            op0=OP.bypass, op1=OP.mult)
                        # keep the PE streaming: next strip's pnum before pz
                        if st + 1 < NSTRIP:
                            pnum = _pnum(ps_num, st + 1)

                        pz = ps_num.tile([P, 2, 512], F32, tag="pnum")
                        for mo in range(2):
                            nc.tensor.matmul(pz[:, mo, :],
                                             wmT[:, 0, mo * P:(mo + 1) * P],
                                             attnT[:, 0, :], start=True, stop=False)
                            nc.tensor.matmul(pz[:, mo, :],
                                             wmT[:, 1, mo * P:(mo + 1) * P],
                                             attnT[:, 1, :], start=False, stop=True)
                        yt = sp.tile([P, 2, 512], BF16, tag="yt")
                        for mo in range(2):
                            nc.scalar.activation(yt[:, mo, :], pz[:, mo, :], AF.Tanh,
                                                 bias=bm_c[:, mo:mo + 1], scale=1.0)

                        # transpose y back to query-partition layout on the PE
                        py = ps_y.tile([P, 4, H], BF16, tag="py")
                        for qq in range(4):
                            for fc in range(2):
                                nc.tensor.transpose(py[:, qq, fc * P:(fc + 1) * P],
                                                    yt[:, fc, qq * P:(qq + 1) * P],
                                                    identb)
                        yout = sp.tile([P, 4, H], F32, tag="yout")
                        nc.vector.tensor_tensor(out=yout, in0=py,
                                                in1=xkb[:, q0:q0 + 4, 0:H], op=OP.add)
                        eng = nc.sync if st % 2 == 0 else nc.scalar
                        eng.dma_start(out=y_r[:, q0:q0 + 4, :], in_=yout)

    nc.compile()
    return nc


def _get_nc():
    if "nc" not in _CACHE:
        _CACHE["nc"] = _build()
    return _CACHE["nc"]


def _in_maps(np_inputs):
    import ml_dtypes
    x = np.asarray(np_inputs["x"], dtype=np.float32)
    w = {}
    for k in ("Wa", "Wb", "Wv", "Wmlp", "ba", "bb", "bv", "bmlp", "Wc", "bc"):
        w[k] = np.ascontiguousarray(np.asarray(np_inputs[k], np.float32))
    in_maps = []
    for c in range(NCORES):
        b, h = divmod(c, 2)
        m = dict(w)
        # full batch of keys, rolled so this core's queries are rows 0:NQ
        m["xk"] = np.ascontiguousarray(
            np.concatenate([x[b, h * NQ:], x[b, :h * NQ]],
                           axis=0).astype(ml_dtypes.bfloat16))
        in_maps.append(m)
    return in_maps


def kernel(x, Wa, ba, Wb, bb, Wv, bv, Wc, bc, Wmlp, bmlp):
    from concourse.bass_utils import run_bass_kernel_spmd

    nc = _get_nc()
    in_maps = _in_maps(dict(x=x, Wa=Wa, ba=ba, Wb=Wb, bb=bb, Wv=Wv, bv=bv,
                            Wc=Wc, bc=bc, Wmlp=Wmlp, bmlp=bmlp))
    res = run_bass_kernel_spmd(nc, in_maps, core_ids=list(range(NCORES)))
    out = np.empty((B, N, H), np.float32)
    for c in range(NCORES):
        b, h = divmod(c, 2)
        out[b, h * NQ:(h + 1) * NQ] = res.results[c]["y"]
    return out
